# revision 23
# baseline (speedup 1.0000x reference)
"""Paged decode attention (nn_Attention_5626407157951) on 8 Trainium2 cores.

Tensor-parallel over heads: each core owns 4 of 32 heads. Per core:
  qkv = hidden @ W_pack[:, own cols]      (bf16 matmuls, fp32 acc)
  rotary(q, k) at pos=hist                (DVE, fp32; host-built cos/sin)
  scores_T[s, (h,pair)] = K_pair^T q      (PE, K stationary bf16, q moving)
  softmax without max-subtraction; new token handled analytically:
      out = (sum_s exp(s)*v_s + e_new*v_new) / (sum_s exp(s) + e_new)
  out_partial = attn @ o_proj[:, own dims].T ; host sums the 8 partials.

Everything is bf16 (2 bytes/elem): the correctness gate is rel_err < 2e-2
and pure-bf16 lands ~7e-3, so no hi/lo error-compensation splits are
needed. This halves HBM traffic vs a 3-byte hi/lo scheme and cuts the
matmul count 3x. Host pre-transposes weights/caches into DMA-friendly
layouts with large contiguous runs per partition.
"""

import math
import os

import ml_dtypes
import numpy as np

import concourse.bass as bass
import concourse.mybir as mybir
import concourse.tile as tile
from concourse.bass_utils import run_bass_kernel_spmd
from concourse.vector_clock import ScopedClock

B = 32          # batch (decode requests)
H = 32          # total heads
HL = 4          # heads per core
D = 128         # head dim
HID = 4096
BS = 64         # cache block size
NBLK = 16       # blocks per request
NCORES = 8
KT = HID // 128         # 32 contraction tiles for qkv proj
PAIRS = NBLK // 2       # 8 block-pairs (128 tokens each) per request
ROPE_BASE = 10000.0
PRE = 5                 # KV prefetch depth (requests ahead)

F32 = mybir.dt.float32
BF = mybir.dt.bfloat16
BF_NP = ml_dtypes.bfloat16
EXP_FN = mybir.ActivationFunctionType.Exp
MUL = mybir.AluOpType.mult
ADD = mybir.AluOpType.add
SUB = mybir.AluOpType.subtract

LAST_RESULTS = None  # test harness peeks at this for profiling info

# ---------------------------------------------------------------------------
# This walrus build accepts very few sync-waits per instruction; the Tile
# kernel-tail drain accumulates one wait per sem lane. Split the waits over
# several drain instructions (all before the barrier, so semantics hold).
_MAX_DRAIN_WAITS = 1


def _patched_drain_and_barrier(self, tick_clock, wait_clock):
    nc = self.nc
    drain_inst = nc.sync.drain()
    wait_clock.add_sem_waits(
        drain_inst.ins, ScopedClock({None: tick_clock.global_clock})
    )
    si = drain_inst.ins.sync_info
    if si is not None and si.on_wait and len(si.on_wait) > _MAX_DRAIN_WAITS:
        waits = list(si.on_wait)
        drain_inst.ins.sync_info = mybir.SyncInfo(
            on_wait=waits[:_MAX_DRAIN_WAITS], on_update=list(si.on_update or [])
        )
        rest = waits[_MAX_DRAIN_WAITS:]
        for i in range(0, len(rest), _MAX_DRAIN_WAITS):
            extra = nc.sync.drain()
            extra.ins.sync_info = mybir.SyncInfo(
                on_wait=rest[i : i + _MAX_DRAIN_WAITS], on_update=[]
            )
    nc.all_engine_barrier()
    popped = nc._tile_sem_poison_stack.pop()
    assert popped is self._sem_poison
    nc.clear_and_free_semaphores(list(self.sems.allocated().values()))
    nc.all_engine_barrier()


tile.TileContext._drain_and_barrier = _patched_drain_and_barrier


def _split_excess_waits(nc, limit=1):
    """Walrus rejects instructions carrying more than ~1 sync wait. Hoist the
    excess onto NoOps inserted just before, on the same engine queue (the
    queue blocks on them first, so semantics are identical)."""
    for fn in nc.m.functions:
        for bb in fn.blocks:
            out = []
            changed = False
            for inst in list(bb.instructions):
                si = getattr(inst, "sync_info", None)
                if si is not None and si.on_wait and len(si.on_wait) > limit:
                    waits = list(si.on_wait)
                    extra, keep = waits[:-limit], waits[-limit:]
                    for i in range(0, len(extra), limit):
                        nop = mybir.InstNoOp(
                            name=nc.get_next_instruction_name(),
                            ins=[], outs=[], engine=inst.engine,
                            sync_info=mybir.SyncInfo(
                                on_wait=extra[i : i + limit], on_update=[]
                            ),
                        )
                        nc.register_instruction(nop)
                        out.append(nop)
                    inst.sync_info = mybir.SyncInfo(
                        on_wait=keep, on_update=list(si.on_update or [])
                    )
                    changed = True
                out.append(inst)
            if changed:
                bb.instructions = out
# ---------------------------------------------------------------------------


def _build_nc(pairs, order):
    """Build the SPMD bass module. `pairs[b]` = number of 128-token cached
    pairs for request b (same on every core; head split is via input data).
    `order[i]` = original request processed in slot i (descending pairs, so
    the tail of the stream is the cheapest compute). The `ident` input is a
    permutation matrix mapping qkv rows (original b) to slot columns; the
    host unpermutes the output rows."""
    nc = bass.Bass()

    def param(name, shape, dt):
        return nc.declare_dram_parameter(name, list(shape), dt, isOutput=False)

    hT = param("hT", [128, KT, B], BF)
    wp = param("wp", [KT, 128, 3 * HL * D], BF)
    wo = param("wo", [HL, 128, HID], BF)
    kc = param("kc", [128, B, PAIRS, HL, 128], BF)   # [d, b, pair, h, s]
    vc = param("vc", [128, B, PAIRS, HL, 128], BF)   # [s, b, pair, h, d]
    cs = param("cs", [B, 4 * HL * D], F32)
    maskp = param("mask", [128, B, HL, PAIRS], F32)  # slot-indexed
    identp = param("ident", [B, B], BF)              # permutation matrix
    out_part = nc.declare_dram_parameter("out_part", [B, HID], F32, isOutput=True)

    HD = HL * D  # 512 local attention dims

    with tile.TileContext(nc) as tc:
        with (
            tc.tile_pool(name="const", bufs=1) as cpool,
            tc.tile_pool(name="work", bufs=1) as wpool,
            tc.tile_pool(name="wtiles", bufs=8) as wtp,
            tc.tile_pool(name="wop", bufs=4) as wop,
            tc.tile_pool(name="kv", bufs=PRE + 1) as kvp,
            tc.tile_pool(name="small", bufs=3) as smp,
        ):
            # ---- constants ----
            ident = cpool.tile([B, B], BF)
            nc.sync.dma_start(out=ident[:], in_=identp[:])
            ones = cpool.tile([128, 1], BF)
            nc.vector.memset(ones[:], 1.0)
            onesf = cpool.tile([1, HL * B], F32)
            nc.vector.memset(onesf[:], 1.0)
            mask_sb = cpool.tile([128, B, HL, PAIRS], F32)
            nc.sync.dma_start(out=mask_sb[:], in_=maskp[:])
            cs_sb = cpool.tile([B, 4 * HD], F32)
            nc.sync.dma_start(out=cs_sb[:], in_=cs[:])
            hT_sb = cpool.tile([128, KT, B], BF)
            nc.sync.dma_start(out=hT_sb[:], in_=hT[:])

            # per-request KV loads ([128, pb, HL, 128] each, one contiguous
            # run per partition in DRAM)
            kv_tiles = {}

            def load_b(i):
                b = order[i]
                pb = pairs[b]
                kt_ = kvp.tile([128, pb, HL, 128], BF, tag="k")
                nc.sync.dma_start(out=kt_[:], in_=kc[:, b, 0:pb, :, :])
                vt_ = kvp.tile([128, pb, HL, 128], BF, tag="v")
                nc.sync.dma_start(out=vt_[:], in_=vc[:, b, 0:pb, :, :])
                kv_tiles[i] = (kt_, vt_)

            for i in range(PRE):
                if pairs[order[i]] > 0:
                    load_b(i)

            # accumulators written per-b, read in the epilogue
            atsb = wpool.tile([128, HL * B], F32)   # cached attn, col h*32+b
            nc.vector.memset(atsb[:], 0.0)
            dnm = wpool.tile([1, HL * B], F32)      # cached denom, col h*32+b
            nc.vector.memset(dnm[:], 0.0)

            # o_proj weights: issued right after the wp stream so they fill
            # the DMA pipe while the (serial) rope/transpose phase runs
            wo_tiles = {}

            def issue_wo(i):
                wot = wop.tile([128, HID], BF, tag="wot")
                nc.sync.dma_start(out=wot[:], in_=wo[i])
                wo_tiles[i] = wot

            with tc.tile_pool(name="psA", bufs=1, space="PSUM") as psA:
                # PE warmup transpose so `ident` is observed by PE before the
                # real (fp32, single-wait-slot) transposes below.
                tp0 = psA.tile([B, B], BF, tag="tp0")
                nc.tensor.transpose(tp0[:], ident[:], ident[:])

                # ---- phase 1: qkv = hidden @ W_pack (bf16) ----
                qkv_ps = psA.tile([B, 3 * HD], F32, tag="qkv")
                for kt in range(KT):
                    wpt = wtp.tile([128, 3 * HD], BF, tag="wpt")
                    nc.sync.dma_start(out=wpt[:], in_=wp[kt])
                    for n in range(3):
                        nc.tensor.matmul(
                            qkv_ps[:, n * HD : (n + 1) * HD],
                            hT_sb[:, kt, :],
                            wpt[:, n * HD : (n + 1) * HD],
                            start=(kt == 0),
                            stop=(kt == KT - 1),
                        )

                for i in range(HL):
                    issue_wo(i)

                qkv_sb = wpool.tile([B, 3 * HD], F32)
                nc.vector.tensor_copy(qkv_sb[:, 0 : 2 * HD], qkv_ps[:, 0 : 2 * HD])
                nc.scalar.copy(qkv_sb[:, 2 * HD :], qkv_ps[:, 2 * HD :])

                # ---- phase 2: rotary (fp32, DVE) + transposes ----
                def rope(src_off, cs_off):
                    src = qkv_sb[:, src_off : src_off + HD]
                    t1 = wpool.tile([B, HD], F32, tag="rope_t1")
                    nc.vector.tensor_tensor(
                        t1[:], src, cs_sb[:, cs_off : cs_off + HD], MUL
                    )
                    sh = wpool.tile([B, HD], F32, tag="rope_sh")
                    sh4 = sh[:].rearrange("b (h d) -> b h d", h=HL)
                    sr4 = qkv_sb[:, src_off : src_off + HD].rearrange(
                        "b (h d) -> b h d", h=HL
                    )
                    nc.scalar.copy(sh4[:, :, 0:64], sr4[:, :, 64:128])
                    nc.scalar.copy(sh4[:, :, 64:128], sr4[:, :, 0:64])
                    nc.vector.tensor_tensor(
                        sh[:], sh[:], cs_sb[:, cs_off + HD : cs_off + 2 * HD], MUL
                    )
                    nc.vector.tensor_tensor(
                        qkv_sb[:, src_off : src_off + HD], t1[:], sh[:], ADD
                    )

                rope(0, 0)
                rope(HD, 2 * HD)

                # bf16 copy of q/k (post-rope) and v for the PE transposes
                qkv_bf = wpool.tile([B, 3 * HD], BF)
                nc.vector.tensor_copy(qkv_bf[:, 0 : 2 * HD], qkv_sb[:, 0 : 2 * HD])
                nc.scalar.copy(qkv_bf[:, 2 * HD :], qkv_sb[:, 2 * HD :])

            # PE transposes (bf16, permuted to slot order by `ident`)
            qT_bf = wpool.tile([128, HL * B], BF)
            vT = wpool.tile([128, HL * B], F32)
            prod = wpool.tile([128, HL * B], BF)
            with tc.tile_pool(name="psT", bufs=2, space="PSUM") as psT:
                for h in range(HL):
                    tpq = psT.tile([128, B], BF, tag="tpq")
                    nc.tensor.transpose(
                        tpq[:], qkv_bf[:, h * D : (h + 1) * D], ident[:]
                    )
                    tpk = psT.tile([128, B], BF, tag="tpk")
                    nc.tensor.transpose(
                        tpk[:], qkv_bf[:, HD + h * D : HD + (h + 1) * D], ident[:]
                    )
                    tpv = psT.tile([128, B], BF, tag="tpv")
                    nc.tensor.transpose(
                        tpv[:], qkv_bf[:, 2 * HD + h * D : 2 * HD + (h + 1) * D],
                        ident[:],
                    )
                    nc.vector.tensor_copy(qT_bf[:, h * B : (h + 1) * B], tpq[:])
                    nc.scalar.copy(vT[:, h * B : (h + 1) * B], tpv[:])
                    # new-token score terms: q_d * k_d (slot order), bf16
                    nc.vector.tensor_tensor(
                        prod[:, h * B : (h + 1) * B],
                        qT_bf[:, h * B : (h + 1) * B], tpk[:], MUL
                    )

                sn_ps = psT.tile([1, HL * B], F32, tag="sn")
                nc.tensor.matmul(sn_ps[:], ones[:], prod[:], start=True, stop=True)
                e_new = wpool.tile([1, HL * B], F32)
                nc.scalar.activation(e_new[:], sn_ps[:], EXP_FN)

            # ---- phase 3: per-request paged attention (slot order) ----
            # Software-pipelined one request ahead: scores(i+1) is issued
            # before attnV(i) so the PE never stalls on the mask->exp->cast
            # round trip through DVE/ACT.
            ph_tiles = {}
            with (
                tc.tile_pool(name="psB", bufs=3, space="PSUM") as psB,
                tc.tile_pool(name="psB2", bufs=2, space="PSUM") as psB2,
            ):
                def do_scores(i):
                    pb = pairs[order[i]]
                    kt_, _ = kv_tiles[i]
                    # scores^T: [128(s), (h, pair)]
                    scp = psB.tile([128, HL, pb], F32, tag="scp")
                    for h in range(HL):
                        qh = qT_bf[:, h * B + i : h * B + i + 1]
                        for p in range(pb):
                            nc.tensor.matmul(
                                scp[:, h, p : p + 1], kt_[:, p, h, :], qh,
                                start=True, stop=True,
                            )
                    # mask + exp -> probs (bf16)
                    tmps = smp.tile([128, HL, pb], F32, tag="tmps")
                    nc.vector.tensor_tensor(
                        tmps[:], scp[:], mask_sb[:, i, :, 0:pb], ADD
                    )
                    expb = smp.tile([128, HL, pb], F32, tag="expb")
                    nc.scalar.activation(expb[:], tmps[:], EXP_FN)
                    ph = smp.tile([128, HL, pb], BF, tag="ph")
                    nc.vector.tensor_copy(ph[:], expb[:])
                    ph_tiles[i] = ph

                def do_attnv(i):
                    pb = pairs[order[i]]
                    _, vt_ = kv_tiles.pop(i)
                    ph = ph_tiles.pop(i)
                    # attn^T[d, h] = sum_s p[s] * V[s, d]
                    atp = psB.tile([128, HL], F32, tag="atp")
                    for h in range(HL):
                        for p in range(pb):
                            nc.tensor.matmul(
                                atp[:, h : h + 1], vt_[:, p, h, :],
                                ph[:, h, p : p + 1],
                                start=(p == 0), stop=(p == pb - 1),
                            )
                    nc.scalar.copy(
                        atsb[:].rearrange("d (h b2) -> d h b2", h=HL)[:, :, i], atp[:]
                    )
                    # denominators: column sums of probs
                    dsp = psB2.tile([1, HL * pb], F32, tag="dsp")
                    nc.tensor.matmul(
                        dsp[:], ones[:],
                        ph[:].rearrange("s h p -> s (h p)"),
                        start=True, stop=True,
                    )
                    nc.vector.reduce_sum(
                        dnm[:].rearrange("o (h b2) -> o h b2", h=HL)[:, :, i],
                        dsp[:].rearrange("o (h p) -> o h p", h=HL),
                        axis=mybir.AxisListType.X,
                    )

                do_scores(0)
                for i in range(B):
                    nxt = i + PRE
                    if nxt < B and nxt not in kv_tiles:
                        load_b(nxt)
                    if i + 1 < B:
                        do_scores(i + 1)
                    do_attnv(i)

            # ---- epilogue: add new token, normalize, project ----
            dtot = wpool.tile([1, HL * B], F32)
            nc.vector.tensor_tensor(dtot[:], dnm[:], e_new[:], ADD)
            rec = wpool.tile([1, HL * B], F32)
            nc.vector.reciprocal(rec[:], dtot[:])
            att = wpool.tile([128, HL * B], F32)
            with tc.tile_pool(name="psD", bufs=1, space="PSUM") as psD:
                # broadcast rows across partitions via K=1 outer products
                ebp = psD.tile([128, HL * B], F32, tag="ebp")
                nc.tensor.matmul(ebp[:], onesf[:], e_new[:], start=True, stop=True)
                rbp = psD.tile([128, HL * B], F32, tag="rbp")
                nc.tensor.matmul(rbp[:], onesf[:], rec[:], start=True, stop=True)

                nc.vector.tensor_tensor(att[:], vT[:], ebp[:], MUL)
                nc.vector.tensor_tensor(att[:], att[:], atsb[:], ADD)
                nc.vector.tensor_tensor(att[:], att[:], rbp[:], MUL)
            at_bf = wpool.tile([128, HL * B], BF)
            nc.vector.tensor_copy(at_bf[:], att[:])

            with tc.tile_pool(name="psC", bufs=3, space="PSUM") as psC:
                outc = wpool.tile([B, HID], F32)
                for n in range(8):
                    opsn = psC.tile([B, 512], F32, tag="ops")
                    for h in range(HL):
                        nc.tensor.matmul(
                            opsn[:],
                            at_bf[:, h * B : (h + 1) * B],
                            wo_tiles[h][:, n * 512 : (n + 1) * 512],
                            start=(h == 0),
                            stop=(h == HL - 1),
                        )
                    oslice = outc[:, n * 512 : (n + 1) * 512]
                    if n % 2:
                        nc.scalar.copy(oslice, opsn[:])
                    else:
                        nc.vector.tensor_copy(oslice, opsn[:])
                nc.sync.dma_start(out=out_part[:], in_=outc[:])

    _split_excess_waits(nc)
    return nc


def _host_prep(hidden, W_pack, o_proj_weight, k_cache, v_cache, hist, block_offsets):
    """Build the 8 per-core input maps (numpy only)."""
    hidden = np.asarray(hidden, np.float32)
    W_pack = np.asarray(W_pack, np.float32)
    o_proj_weight = np.asarray(o_proj_weight, np.float32)
    k_cache = np.asarray(k_cache, np.float32)
    v_cache = np.asarray(v_cache, np.float32)
    hist = np.asarray(hist, np.int64)
    block_offsets = np.asarray(block_offsets, np.int64)

    pairs = [int((h + 127) // 128) for h in hist]
    # slot order: descending pairs so the end of the KV stream (which is no
    # longer overlapped with DMA) carries the cheapest compute
    order = sorted(range(B), key=lambda b: (-pairs[b], b))

    # rope tables, scale folded into the q tables
    inv_freq = 1.0 / (ROPE_BASE ** (np.arange(0, D, 2, dtype=np.float32) / D))
    ang = hist.astype(np.float32)[:, None] * inv_freq[None, :]        # [B, 64]
    cos128 = np.concatenate([np.cos(ang), np.cos(ang)], -1)           # [B, 128]
    sin128 = np.concatenate([np.sin(ang), np.sin(ang)], -1)
    sign = np.concatenate([-np.ones(64), np.ones(64)]).astype(np.float32)
    sc = 1.0 / math.sqrt(D)
    tile_h = lambda x: np.tile(x, (1, HL)).astype(np.float32)         # [B, 512]
    cs = np.concatenate(
        [tile_h(cos128 * sc), tile_h(sin128 * sign * sc),
         tile_h(cos128), tile_h(sin128 * sign)], -1,
    )                                                                 # [B, 2048]

    # additive mask over the loaded pairs: position 128*p + s valid iff < hist
    s_idx = np.arange(128)[:, None, None]                             # s
    p_idx = np.arange(PAIRS)[None, None, :]                           # pair
    pos = p_idx * 128 + s_idx                                         # [128,1,8]
    valid = pos < hist[None, :, None]                                 # [128,B,8]
    mask = np.where(valid, 0.0, -1e30).astype(np.float32)             # [128,B,8]
    mask = np.repeat(mask[:, :, None, :], HL, axis=2)                 # [128,B,4,8]
    mask = mask[:, order]                                             # slot-indexed

    hT = np.ascontiguousarray(hidden.T)                               # [4096, 32]
    hT_bf = np.ascontiguousarray(
        hT.astype(BF_NP).reshape(KT, 128, B).transpose(1, 0, 2)
    )                                                                 # [128, KT, B]

    # gather caches via the block table (b-major), slice heads per core
    k_all = k_cache[block_offsets.reshape(-1)]                        # [512,64,32,128]
    v_all = v_cache[block_offsets.reshape(-1)]

    # permutation matrix: column slot i picks original request order[i]
    ident = np.zeros((B, B), dtype=BF_NP)
    ident[np.asarray(order), np.arange(B)] = 1.0

    in_maps = []
    for c in range(NCORES):
        h0 = c * HL
        qcols = np.arange(h0 * D, (h0 + HL) * D)
        wp_c = np.concatenate(
            [W_pack[:, qcols], W_pack[:, HID + qcols], W_pack[:, 2 * HID + qcols]],
            axis=1,
        )                                                             # [4096, 1536]
        wp_bf = wp_c.astype(BF_NP).reshape(KT, 128, 3 * HL * D)

        wo_c = np.ascontiguousarray(o_proj_weight[:, qcols].T)        # [512, 4096]
        wo_bf = wo_c.astype(BF_NP).reshape(HL, 128, HID)

        kc = k_all[:, :, h0 : h0 + HL, :]                             # [512,64,4,128]
        vc = v_all[:, :, h0 : h0 + HL, :]
        # K: [128(d), B, PAIRS, HL, 128(s)]
        kc5 = kc.reshape(B, PAIRS, 2, BS, HL, D)
        kT_c = np.ascontiguousarray(
            kc5.transpose(5, 0, 1, 4, 2, 3).reshape(D, B, PAIRS, HL, 128)
        ).astype(BF_NP)
        # V: [128(s), B, PAIRS, HL, 128(d)]
        vc5 = vc.reshape(B, PAIRS, 2, BS, HL, D)
        v_c = np.ascontiguousarray(
            vc5.transpose(2, 3, 0, 1, 4, 5).reshape(128, B, PAIRS, HL, D)
        ).astype(BF_NP)

        in_maps.append({
            "hT": hT_bf, "wp": wp_bf, "wo": wo_bf,
            "kc": kT_c, "vc": v_c,
            "cs": cs, "mask": mask, "ident": ident,
        })
    return pairs, order, in_maps


def kernel(hidden_states, W_pack, o_proj_weight, k_cache, v_cache,
           history_lengths, block_offsets):
    global LAST_RESULTS
    pairs, order, in_maps = _host_prep(
        hidden_states, W_pack, o_proj_weight, k_cache, v_cache,
        history_lengths, block_offsets,
    )
    nc = _build_nc(pairs, order)
    trace = bool(int(os.environ.get("KERNEL_TRACE", "0")))
    res = run_bass_kernel_spmd(nc, in_maps, list(range(NCORES)), trace=trace)
    LAST_RESULTS = res
    acc = np.zeros((B, HID), np.float32)
    for c in range(NCORES):
        acc += res.results[c]["out_part"]
    out = np.zeros((B, HID), np.float32)
    out[np.asarray(order)] = acc                   # slot rows -> original rows
    return out


# revision 26
# speedup vs baseline: 1.0027x; 1.0027x over previous
"""Paged decode attention (nn_Attention_5626407157951) on 8 Trainium2 cores.

Tensor-parallel over heads: each core owns 4 of 32 heads. Per core:
  qkv = hidden @ W_pack[:, own cols]      (bf16 matmuls, fp32 acc)
  rotary(q, k) at pos=hist                (DVE, fp32; host-built cos/sin)
  scores_T[s, (h,pair)] = K_pair^T q      (PE, K stationary bf16, q moving)
  softmax without max-subtraction; new token handled analytically:
      out = (sum_s exp(s)*v_s + e_new*v_new) / (sum_s exp(s) + e_new)
  out_partial = attn @ o_proj[:, own dims].T ; host sums the 8 partials.

Everything is bf16 (2 bytes/elem): the correctness gate is rel_err < 2e-2
and pure-bf16 lands ~7e-3, so no hi/lo error-compensation splits are
needed. This halves HBM traffic vs a 3-byte hi/lo scheme and cuts the
matmul count 3x. Host pre-transposes weights/caches into DMA-friendly
layouts with large contiguous runs per partition.
"""

import math
import os

import ml_dtypes
import numpy as np

import concourse.bass as bass
import concourse.mybir as mybir
import concourse.tile as tile
from concourse.bass_utils import run_bass_kernel_spmd
from concourse.vector_clock import ScopedClock

B = 32          # batch (decode requests)
H = 32          # total heads
HL = 4          # heads per core
D = 128         # head dim
HID = 4096
BS = 64         # cache block size
NBLK = 16       # blocks per request
NCORES = 8
KT = HID // 128         # 32 contraction tiles for qkv proj
PAIRS = NBLK // 2       # 8 block-pairs (128 tokens each) per request
ROPE_BASE = 10000.0
PRE = 5                 # KV prefetch depth (requests ahead)

F32 = mybir.dt.float32
BF = mybir.dt.bfloat16
BF_NP = ml_dtypes.bfloat16
EXP_FN = mybir.ActivationFunctionType.Exp
COPY_FN = mybir.ActivationFunctionType.Copy
MUL = mybir.AluOpType.mult
ADD = mybir.AluOpType.add
SUB = mybir.AluOpType.subtract

LAST_RESULTS = None  # test harness peeks at this for profiling info

# ---------------------------------------------------------------------------
# This walrus build accepts very few sync-waits per instruction; the Tile
# kernel-tail drain accumulates one wait per sem lane. Split the waits over
# several drain instructions (all before the barrier, so semantics hold).
_MAX_DRAIN_WAITS = 1


def _patched_drain_and_barrier(self, tick_clock, wait_clock):
    nc = self.nc
    drain_inst = nc.sync.drain()
    wait_clock.add_sem_waits(
        drain_inst.ins, ScopedClock({None: tick_clock.global_clock})
    )
    si = drain_inst.ins.sync_info
    if si is not None and si.on_wait and len(si.on_wait) > _MAX_DRAIN_WAITS:
        waits = list(si.on_wait)
        drain_inst.ins.sync_info = mybir.SyncInfo(
            on_wait=waits[:_MAX_DRAIN_WAITS], on_update=list(si.on_update or [])
        )
        rest = waits[_MAX_DRAIN_WAITS:]
        for i in range(0, len(rest), _MAX_DRAIN_WAITS):
            extra = nc.sync.drain()
            extra.ins.sync_info = mybir.SyncInfo(
                on_wait=rest[i : i + _MAX_DRAIN_WAITS], on_update=[]
            )
    nc.all_engine_barrier()
    popped = nc._tile_sem_poison_stack.pop()
    assert popped is self._sem_poison
    nc.clear_and_free_semaphores(list(self.sems.allocated().values()))
    nc.all_engine_barrier()


tile.TileContext._drain_and_barrier = _patched_drain_and_barrier


def _split_excess_waits(nc, limit=1):
    """Walrus rejects instructions carrying more than ~1 sync wait. Hoist the
    excess onto NoOps inserted just before, on the same engine queue (the
    queue blocks on them first, so semantics are identical)."""
    for fn in nc.m.functions:
        for bb in fn.blocks:
            out = []
            changed = False
            for inst in list(bb.instructions):
                si = getattr(inst, "sync_info", None)
                if si is not None and si.on_wait and len(si.on_wait) > limit:
                    waits = list(si.on_wait)
                    extra, keep = waits[:-limit], waits[-limit:]
                    for i in range(0, len(extra), limit):
                        nop = mybir.InstNoOp(
                            name=nc.get_next_instruction_name(),
                            ins=[], outs=[], engine=inst.engine,
                            sync_info=mybir.SyncInfo(
                                on_wait=extra[i : i + limit], on_update=[]
                            ),
                        )
                        nc.register_instruction(nop)
                        out.append(nop)
                    inst.sync_info = mybir.SyncInfo(
                        on_wait=keep, on_update=list(si.on_update or [])
                    )
                    changed = True
                out.append(inst)
            if changed:
                bb.instructions = out
# ---------------------------------------------------------------------------


def _build_nc(pairs, order, rtail):
    """Build the SPMD bass module. `pairs[b]` = number of 128-token cached
    pairs for request b (same on every core; head split is via input data).
    `order[i]` = original request processed in slot i (descending pairs, so
    the tail of the stream is the cheapest compute). The `ident` input is a
    permutation matrix mapping qkv rows (original b) to slot columns; the
    host unpermutes the output rows."""
    nc = bass.Bass()

    def param(name, shape, dt):
        return nc.declare_dram_parameter(name, list(shape), dt, isOutput=False)

    hT = param("hT", [128, KT, B], BF)
    wp = param("wp", [KT, 128, 3 * HL * D], BF)
    wo = param("wo", [HL, 128, HID], BF)
    kc = param("kc", [128, B, PAIRS, HL, 128], BF)   # [d, b, pair, h, s]
    vc = param("vc", [128, B, PAIRS, HL, 128], BF)   # [s, b, pair, h, d]
    cs = param("cs", [B, 4 * HL * D], F32)
    zmaskp = param("zmask", [128, B], F32)            # 1 iff row s < hist (last pair)
    identp = param("ident", [B, B], BF)              # permutation matrix
    out_part = nc.declare_dram_parameter("out_part", [B, HID], F32, isOutput=True)

    HD = HL * D  # 512 local attention dims

    with tile.TileContext(nc) as tc:
        with (
            tc.tile_pool(name="const", bufs=1) as cpool,
            tc.tile_pool(name="work", bufs=1) as wpool,
            tc.tile_pool(name="wtiles", bufs=8) as wtp,
            tc.tile_pool(name="wop", bufs=4) as wop,
            tc.tile_pool(name="kv", bufs=PRE + 1) as kvp,
            tc.tile_pool(name="small", bufs=3) as smp,
        ):
            # ---- constants ----
            ident = cpool.tile([B, B], BF)
            nc.sync.dma_start(out=ident[:], in_=identp[:])
            ones = cpool.tile([128, 1], BF)
            nc.vector.memset(ones[:], 1.0)
            onesf = cpool.tile([1, HL * B], F32)
            nc.vector.memset(onesf[:], 1.0)
            zmask = cpool.tile([128, B], F32)
            nc.sync.dma_start(out=zmask[:], in_=zmaskp[:])
            cs_sb = cpool.tile([B, 4 * HD], F32)
            nc.sync.dma_start(out=cs_sb[:], in_=cs[:])
            hT_sb = cpool.tile([128, KT, B], BF)
            nc.sync.dma_start(out=hT_sb[:], in_=hT[:])

            # per-request KV loads ([128, pb, HL, 128] each, one contiguous
            # run per partition in DRAM)
            kv_tiles = {}

            def load_b(i):
                b = order[i]
                pb = pairs[b]
                kt_ = kvp.tile([128, pb, HL, 128], BF, tag="k")
                nc.sync.dma_start(out=kt_[:], in_=kc[:, b, 0:pb, :, :])
                vt_ = kvp.tile([128, pb, HL, 128], BF, tag="v")
                nc.sync.dma_start(out=vt_[:], in_=vc[:, b, 0:pb, :, :])
                kv_tiles[i] = (kt_, vt_)

            for i in range(PRE):
                if pairs[order[i]] > 0:
                    load_b(i)

            # accumulators written per-b, read in the epilogue
            atsb = wpool.tile([128, HL * B], F32)   # cached attn, col h*32+b
            nc.vector.memset(atsb[:], 0.0)
            dnm = wpool.tile([1, HL * B], F32)      # cached denom, col h*32+b
            nc.vector.memset(dnm[:], 0.0)

            # o_proj weights: issued right after the wp stream so they fill
            # the DMA pipe while the (serial) rope/transpose phase runs
            wo_tiles = {}

            def issue_wo(i):
                wot = wop.tile([128, HID], BF, tag="wot")
                nc.sync.dma_start(out=wot[:], in_=wo[i])
                wo_tiles[i] = wot

            with tc.tile_pool(name="psA", bufs=1, space="PSUM") as psA:
                # PE warmup transpose so `ident` is observed by PE before the
                # real (fp32, single-wait-slot) transposes below.
                tp0 = psA.tile([B, B], BF, tag="tp0")
                nc.tensor.transpose(tp0[:], ident[:], ident[:])

                # ---- phase 1: qkv = hidden @ W_pack (bf16) ----
                qkv_ps = psA.tile([B, 3 * HD], F32, tag="qkv")
                for kt in range(KT):
                    wpt = wtp.tile([128, 3 * HD], BF, tag="wpt")
                    nc.sync.dma_start(out=wpt[:], in_=wp[kt])
                    for n in range(3):
                        nc.tensor.matmul(
                            qkv_ps[:, n * HD : (n + 1) * HD],
                            hT_sb[:, kt, :],
                            wpt[:, n * HD : (n + 1) * HD],
                            start=(kt == 0),
                            stop=(kt == KT - 1),
                        )

                for i in range(HL):
                    issue_wo(i)

                qkv_sb = wpool.tile([B, 3 * HD], F32)
                nc.vector.tensor_copy(qkv_sb[:, 0 : 2 * HD], qkv_ps[:, 0 : 2 * HD])
                nc.scalar.copy(qkv_sb[:, 2 * HD :], qkv_ps[:, 2 * HD :])

                # ---- phase 2: rotary (fp32, DVE) + transposes ----
                def rope(src_off, cs_off):
                    src = qkv_sb[:, src_off : src_off + HD]
                    t1 = wpool.tile([B, HD], F32, tag="rope_t1")
                    nc.vector.tensor_tensor(
                        t1[:], src, cs_sb[:, cs_off : cs_off + HD], MUL
                    )
                    sh = wpool.tile([B, HD], F32, tag="rope_sh")
                    sh4 = sh[:].rearrange("b (h d) -> b h d", h=HL)
                    sr4 = qkv_sb[:, src_off : src_off + HD].rearrange(
                        "b (h d) -> b h d", h=HL
                    )
                    nc.scalar.copy(sh4[:, :, 0:64], sr4[:, :, 64:128])
                    nc.scalar.copy(sh4[:, :, 64:128], sr4[:, :, 0:64])
                    nc.vector.tensor_tensor(
                        sh[:], sh[:], cs_sb[:, cs_off + HD : cs_off + 2 * HD], MUL
                    )
                    nc.vector.tensor_tensor(
                        qkv_sb[:, src_off : src_off + HD], t1[:], sh[:], ADD
                    )

                rope(0, 0)
                rope(HD, 2 * HD)

                # bf16 copy of q/k (post-rope) and v for the PE transposes
                qkv_bf = wpool.tile([B, 3 * HD], BF)
                nc.vector.tensor_copy(qkv_bf[:, 0 : 2 * HD], qkv_sb[:, 0 : 2 * HD])
                nc.scalar.copy(qkv_bf[:, 2 * HD :], qkv_sb[:, 2 * HD :])

            # PE transposes (bf16, permuted to slot order by `ident`)
            qT_bf = wpool.tile([128, HL * B], BF)
            vT = wpool.tile([128, HL * B], F32)
            prod = wpool.tile([128, HL * B], BF)
            with tc.tile_pool(name="psT", bufs=2, space="PSUM") as psT:
                for h in range(HL):
                    tpq = psT.tile([128, B], BF, tag="tpq")
                    nc.tensor.transpose(
                        tpq[:], qkv_bf[:, h * D : (h + 1) * D], ident[:]
                    )
                    tpk = psT.tile([128, B], BF, tag="tpk")
                    nc.tensor.transpose(
                        tpk[:], qkv_bf[:, HD + h * D : HD + (h + 1) * D], ident[:]
                    )
                    tpv = psT.tile([128, B], BF, tag="tpv")
                    nc.tensor.transpose(
                        tpv[:], qkv_bf[:, 2 * HD + h * D : 2 * HD + (h + 1) * D],
                        ident[:],
                    )
                    nc.vector.tensor_copy(qT_bf[:, h * B : (h + 1) * B], tpq[:])
                    nc.scalar.copy(vT[:, h * B : (h + 1) * B], tpv[:])
                    # new-token score terms: q_d * k_d (slot order), bf16
                    nc.vector.tensor_tensor(
                        prod[:, h * B : (h + 1) * B],
                        qT_bf[:, h * B : (h + 1) * B], tpk[:], MUL
                    )

                sn_ps = psT.tile([1, HL * B], F32, tag="sn")
                nc.tensor.matmul(sn_ps[:], ones[:], prod[:], start=True, stop=True)
                e_new = wpool.tile([1, HL * B], F32)
                nc.scalar.activation(e_new[:], sn_ps[:], EXP_FN)

            # ---- phase 3: per-request paged attention (slot order) ----
            # Software-pipelined one request ahead: scores(i+1) is issued
            # before attnV(i) so the PE never stalls on the mask->exp->cast
            # round trip through DVE/ACT.
            ph_tiles = {}
            with (
                tc.tile_pool(name="psB", bufs=3, space="PSUM") as psB,
                tc.tile_pool(name="psB2", bufs=2, space="PSUM") as psB2,
            ):
                def do_scores(i):
                    pb = pairs[order[i]]
                    r = rtail[i]          # valid rows in the last pair
                    kt_, _ = kv_tiles[i]
                    # scores^T: [128(s), (h, pair)]
                    scp = psB.tile([128, HL, pb], F32, tag="scp")
                    for h in range(HL):
                        qh = qT_bf[:, h * B + i : h * B + i + 1]
                        for p in range(pb):
                            nc.tensor.matmul(
                                scp[:, h, p : p + 1], kt_[:, p, h, :], qh,
                                start=True, stop=True,
                            )
                    # probs = exp(scores) in bf16 straight off PSUM; rows
                    # >= hist in the last pair are zeroed by an ACT copy with
                    # a per-partition 0/1 scale (same engine, no extra hop)
                    ph = smp.tile([128, HL, pb], BF, tag="ph")
                    nc.scalar.activation(ph[:], scp[:], EXP_FN)
                    if r < 128:
                        nc.scalar.activation(
                            ph[:, :, pb - 1], ph[:, :, pb - 1], COPY_FN,
                            scale=zmask[:, i : i + 1],
                        )
                    ph_tiles[i] = ph

                def do_attnv(i):
                    pb = pairs[order[i]]
                    _, vt_ = kv_tiles.pop(i)
                    ph = ph_tiles.pop(i)
                    # attn^T[d, h] = sum_s p[s] * V[s, d]
                    atp = psB.tile([128, HL], F32, tag="atp")
                    for h in range(HL):
                        for p in range(pb):
                            nc.tensor.matmul(
                                atp[:, h : h + 1], vt_[:, p, h, :],
                                ph[:, h, p : p + 1],
                                start=(p == 0), stop=(p == pb - 1),
                            )
                    nc.scalar.copy(
                        atsb[:].rearrange("d (h b2) -> d h b2", h=HL)[:, :, i], atp[:]
                    )
                    # denominators: column sums of probs
                    dsp = psB2.tile([1, HL * pb], F32, tag="dsp")
                    nc.tensor.matmul(
                        dsp[:], ones[:],
                        ph[:].rearrange("s h p -> s (h p)"),
                        start=True, stop=True,
                    )
                    nc.vector.reduce_sum(
                        dnm[:].rearrange("o (h b2) -> o h b2", h=HL)[:, :, i],
                        dsp[:].rearrange("o (h p) -> o h p", h=HL),
                        axis=mybir.AxisListType.X,
                    )

                do_scores(0)
                do_scores(1)
                for i in range(B):
                    nxt = i + PRE
                    if nxt < B and nxt not in kv_tiles:
                        load_b(nxt)
                    if i + 2 < B:
                        do_scores(i + 2)
                    do_attnv(i)

            # ---- epilogue: add new token, normalize, project ----
            dtot = wpool.tile([1, HL * B], F32)
            nc.vector.tensor_tensor(dtot[:], dnm[:], e_new[:], ADD)
            rec = wpool.tile([1, HL * B], F32)
            nc.vector.reciprocal(rec[:], dtot[:])
            att = wpool.tile([128, HL * B], F32)
            with tc.tile_pool(name="psD", bufs=1, space="PSUM") as psD:
                # broadcast rows across partitions via K=1 outer products
                ebp = psD.tile([128, HL * B], F32, tag="ebp")
                nc.tensor.matmul(ebp[:], onesf[:], e_new[:], start=True, stop=True)
                rbp = psD.tile([128, HL * B], F32, tag="rbp")
                nc.tensor.matmul(rbp[:], onesf[:], rec[:], start=True, stop=True)

                nc.vector.tensor_tensor(att[:], vT[:], ebp[:], MUL)
                nc.vector.tensor_tensor(att[:], att[:], atsb[:], ADD)
                nc.vector.tensor_tensor(att[:], att[:], rbp[:], MUL)
            at_bf = wpool.tile([128, HL * B], BF)
            nc.vector.tensor_copy(at_bf[:], att[:])

            with tc.tile_pool(name="psC", bufs=3, space="PSUM") as psC:
                outc = wpool.tile([B, HID], F32)
                for n in range(8):
                    opsn = psC.tile([B, 512], F32, tag="ops")
                    for h in range(HL):
                        nc.tensor.matmul(
                            opsn[:],
                            at_bf[:, h * B : (h + 1) * B],
                            wo_tiles[h][:, n * 512 : (n + 1) * 512],
                            start=(h == 0),
                            stop=(h == HL - 1),
                        )
                    oslice = outc[:, n * 512 : (n + 1) * 512]
                    if n % 2:
                        nc.scalar.copy(oslice, opsn[:])
                    else:
                        nc.vector.tensor_copy(oslice, opsn[:])
                nc.sync.dma_start(out=out_part[:], in_=outc[:])

    _split_excess_waits(nc)
    return nc


def _host_prep(hidden, W_pack, o_proj_weight, k_cache, v_cache, hist, block_offsets):
    """Build the 8 per-core input maps (numpy only)."""
    hidden = np.asarray(hidden, np.float32)
    W_pack = np.asarray(W_pack, np.float32)
    o_proj_weight = np.asarray(o_proj_weight, np.float32)
    k_cache = np.asarray(k_cache, np.float32)
    v_cache = np.asarray(v_cache, np.float32)
    hist = np.asarray(hist, np.int64)
    block_offsets = np.asarray(block_offsets, np.int64)

    pairs = [int((h + 127) // 128) for h in hist]
    # slot order: descending pairs so the end of the KV stream (which is no
    # longer overlapped with DMA) carries the cheapest compute
    order = sorted(range(B), key=lambda b: (-pairs[b], b))

    # rope tables, scale folded into the q tables
    inv_freq = 1.0 / (ROPE_BASE ** (np.arange(0, D, 2, dtype=np.float32) / D))
    ang = hist.astype(np.float32)[:, None] * inv_freq[None, :]        # [B, 64]
    cos128 = np.concatenate([np.cos(ang), np.cos(ang)], -1)           # [B, 128]
    sin128 = np.concatenate([np.sin(ang), np.sin(ang)], -1)
    sign = np.concatenate([-np.ones(64), np.ones(64)]).astype(np.float32)
    sc = 1.0 / math.sqrt(D)
    tile_h = lambda x: np.tile(x, (1, HL)).astype(np.float32)         # [B, 512]
    cs = np.concatenate(
        [tile_h(cos128 * sc), tile_h(sin128 * sign * sc),
         tile_h(cos128), tile_h(sin128 * sign)], -1,
    )                                                                 # [B, 2048]

    # valid rows in the last loaded pair, per slot (positions < hist)
    rtail = [int(hist[b]) - 128 * (pairs[b] - 1) for b in order]
    zmask = np.zeros((128, B), dtype=np.float32)
    for i, r in enumerate(rtail):
        zmask[:r, i] = 1.0

    hT = np.ascontiguousarray(hidden.T)                               # [4096, 32]
    hT_bf = np.ascontiguousarray(
        hT.astype(BF_NP).reshape(KT, 128, B).transpose(1, 0, 2)
    )                                                                 # [128, KT, B]

    # gather caches via the block table (b-major), slice heads per core
    k_all = k_cache[block_offsets.reshape(-1)]                        # [512,64,32,128]
    v_all = v_cache[block_offsets.reshape(-1)]

    # permutation matrix: column slot i picks original request order[i]
    ident = np.zeros((B, B), dtype=BF_NP)
    ident[np.asarray(order), np.arange(B)] = 1.0

    in_maps = []
    for c in range(NCORES):
        h0 = c * HL
        qcols = np.arange(h0 * D, (h0 + HL) * D)
        wp_c = np.concatenate(
            [W_pack[:, qcols], W_pack[:, HID + qcols], W_pack[:, 2 * HID + qcols]],
            axis=1,
        )                                                             # [4096, 1536]
        wp_bf = wp_c.astype(BF_NP).reshape(KT, 128, 3 * HL * D)

        wo_c = np.ascontiguousarray(o_proj_weight[:, qcols].T)        # [512, 4096]
        wo_bf = wo_c.astype(BF_NP).reshape(HL, 128, HID)

        kc = k_all[:, :, h0 : h0 + HL, :]                             # [512,64,4,128]
        vc = v_all[:, :, h0 : h0 + HL, :]
        # K: [128(d), B, PAIRS, HL, 128(s)]
        kc5 = kc.reshape(B, PAIRS, 2, BS, HL, D)
        kT_c = np.ascontiguousarray(
            kc5.transpose(5, 0, 1, 4, 2, 3).reshape(D, B, PAIRS, HL, 128)
        ).astype(BF_NP)
        # V: [128(s), B, PAIRS, HL, 128(d)]
        vc5 = vc.reshape(B, PAIRS, 2, BS, HL, D)
        v_c = np.ascontiguousarray(
            vc5.transpose(2, 3, 0, 1, 4, 5).reshape(128, B, PAIRS, HL, D)
        ).astype(BF_NP)

        in_maps.append({
            "hT": hT_bf, "wp": wp_bf, "wo": wo_bf,
            "kc": kT_c, "vc": v_c,
            "cs": cs, "zmask": zmask, "ident": ident,
        })
    return pairs, order, rtail, in_maps


def kernel(hidden_states, W_pack, o_proj_weight, k_cache, v_cache,
           history_lengths, block_offsets):
    global LAST_RESULTS
    pairs, order, rtail, in_maps = _host_prep(
        hidden_states, W_pack, o_proj_weight, k_cache, v_cache,
        history_lengths, block_offsets,
    )
    nc = _build_nc(pairs, order, rtail)
    trace = bool(int(os.environ.get("KERNEL_TRACE", "0")))
    res = run_bass_kernel_spmd(nc, in_maps, list(range(NCORES)), trace=trace)
    LAST_RESULTS = res
    acc = np.zeros((B, HID), np.float32)
    for c in range(NCORES):
        acc += res.results[c]["out_part"]
    out = np.zeros((B, HID), np.float32)
    out[np.asarray(order)] = acc                   # slot rows -> original rows
    return out


# revision 27
# speedup vs baseline: 1.0326x; 1.0299x over previous
"""Paged decode attention (nn_Attention_5626407157951) on 8 Trainium2 cores.

Tensor-parallel over heads: each core owns 4 of 32 heads. Per core:
  qkv = hidden @ W_pack[:, own cols]      (bf16 matmuls, fp32 acc)
  rotary(q, k) at pos=hist                (DVE, fp32; host-built cos/sin)
  scores_T[s, (h,pair)] = K_pair^T q      (PE, K stationary bf16, q moving)
  softmax without max-subtraction; new token handled analytically:
      out = (sum_s exp(s)*v_s + e_new*v_new) / (sum_s exp(s) + e_new)
  out_partial = attn @ o_proj[:, own dims].T ; host sums the 8 partials.

Everything is bf16 (2 bytes/elem): the correctness gate is rel_err < 2e-2
and pure-bf16 lands ~7e-3, so no hi/lo error-compensation splits are
needed. This halves HBM traffic vs a 3-byte hi/lo scheme and cuts the
matmul count 3x. Host pre-transposes weights/caches into DMA-friendly
layouts with large contiguous runs per partition.
"""

import math
import os

import ml_dtypes
import numpy as np

import concourse.bass as bass
import concourse.mybir as mybir
import concourse.tile as tile
from concourse.bass_utils import run_bass_kernel_spmd
from concourse.vector_clock import ScopedClock

B = 32          # batch (decode requests)
H = 32          # total heads
HL = 4          # heads per core
D = 128         # head dim
HID = 4096
BS = 64         # cache block size
NBLK = 16       # blocks per request
NCORES = 8
KT = HID // 128         # 32 contraction tiles for qkv proj
PAIRS = NBLK // 2       # 8 block-pairs (128 tokens each) per request
ROPE_BASE = 10000.0
PRE = 5                 # KV prefetch depth (requests ahead)

F32 = mybir.dt.float32
BF = mybir.dt.bfloat16
BF_NP = ml_dtypes.bfloat16
EXP_FN = mybir.ActivationFunctionType.Exp
COPY_FN = mybir.ActivationFunctionType.Copy
MUL = mybir.AluOpType.mult
ADD = mybir.AluOpType.add
SUB = mybir.AluOpType.subtract

LAST_RESULTS = None  # test harness peeks at this for profiling info

# ---------------------------------------------------------------------------
# This walrus build accepts very few sync-waits per instruction; the Tile
# kernel-tail drain accumulates one wait per sem lane. Split the waits over
# several drain instructions (all before the barrier, so semantics hold).
_MAX_DRAIN_WAITS = 1


def _patched_drain_and_barrier(self, tick_clock, wait_clock):
    nc = self.nc
    drain_inst = nc.sync.drain()
    wait_clock.add_sem_waits(
        drain_inst.ins, ScopedClock({None: tick_clock.global_clock})
    )
    si = drain_inst.ins.sync_info
    if si is not None and si.on_wait and len(si.on_wait) > _MAX_DRAIN_WAITS:
        waits = list(si.on_wait)
        drain_inst.ins.sync_info = mybir.SyncInfo(
            on_wait=waits[:_MAX_DRAIN_WAITS], on_update=list(si.on_update or [])
        )
        rest = waits[_MAX_DRAIN_WAITS:]
        for i in range(0, len(rest), _MAX_DRAIN_WAITS):
            extra = nc.sync.drain()
            extra.ins.sync_info = mybir.SyncInfo(
                on_wait=rest[i : i + _MAX_DRAIN_WAITS], on_update=[]
            )
    nc.all_engine_barrier()
    popped = nc._tile_sem_poison_stack.pop()
    assert popped is self._sem_poison
    nc.clear_and_free_semaphores(list(self.sems.allocated().values()))
    nc.all_engine_barrier()


tile.TileContext._drain_and_barrier = _patched_drain_and_barrier


def _split_excess_waits(nc, limit=1):
    """Walrus rejects instructions carrying more than ~1 sync wait. Hoist the
    excess onto NoOps inserted just before, on the same engine queue (the
    queue blocks on them first, so semantics are identical)."""
    for fn in nc.m.functions:
        for bb in fn.blocks:
            out = []
            changed = False
            for inst in list(bb.instructions):
                si = getattr(inst, "sync_info", None)
                if si is not None and si.on_wait and len(si.on_wait) > limit:
                    waits = list(si.on_wait)
                    extra, keep = waits[:-limit], waits[-limit:]
                    for i in range(0, len(extra), limit):
                        nop = mybir.InstNoOp(
                            name=nc.get_next_instruction_name(),
                            ins=[], outs=[], engine=inst.engine,
                            sync_info=mybir.SyncInfo(
                                on_wait=extra[i : i + limit], on_update=[]
                            ),
                        )
                        nc.register_instruction(nop)
                        out.append(nop)
                    inst.sync_info = mybir.SyncInfo(
                        on_wait=keep, on_update=list(si.on_update or [])
                    )
                    changed = True
                out.append(inst)
            if changed:
                bb.instructions = out
# ---------------------------------------------------------------------------


def _build_nc(pairs, order, rtail):
    """Build the SPMD bass module. `pairs[b]` = number of 128-token cached
    pairs for request b (same on every core; head split is via input data).
    `order[i]` = original request processed in slot i (descending pairs, so
    the tail of the stream is the cheapest compute). The `ident` input is a
    permutation matrix mapping qkv rows (original b) to slot columns; the
    host unpermutes the output rows."""
    nc = bass.Bass()

    def param(name, shape, dt):
        return nc.declare_dram_parameter(name, list(shape), dt, isOutput=False)

    hT = param("hT", [128, KT, B], BF)
    wp = param("wp", [KT, 128, 3 * HL * D], BF)
    wo = param("wo", [HL, 128, HID], BF)
    kc = param("kc", [128, B, PAIRS, HL, 128], BF)   # [d, b, pair, h, s]
    vc = param("vc", [128, B, PAIRS, HL, 128], BF)   # [s, b, pair, h, d]
    cs = param("cs", [B, 4 * HL * D], F32)
    zmaskp = param("zmask", [128, B], F32)            # 1 iff row s < hist (last pair)
    identp = param("ident", [B, B], BF)              # permutation matrix
    out_part = nc.declare_dram_parameter("out_part", [B, HID], F32, isOutput=True)

    HD = HL * D  # 512 local attention dims

    with tile.TileContext(nc) as tc:
        with (
            tc.tile_pool(name="const", bufs=1) as cpool,
            tc.tile_pool(name="work", bufs=1) as wpool,
            tc.tile_pool(name="wtiles", bufs=12) as wtp,
            tc.tile_pool(name="wop", bufs=4) as wop,
            tc.tile_pool(name="kv", bufs=PRE + 1) as kvp,
            tc.tile_pool(name="small", bufs=3) as smp,
        ):
            # ---- constants ----
            ident = cpool.tile([B, B], BF)
            nc.sync.dma_start(out=ident[:], in_=identp[:])
            ones = cpool.tile([128, 1], BF)
            nc.vector.memset(ones[:], 1.0)
            onesf = cpool.tile([1, HL * B], F32)
            nc.vector.memset(onesf[:], 1.0)
            zmask = cpool.tile([128, B], F32)
            nc.sync.dma_start(out=zmask[:], in_=zmaskp[:])
            cs_sb = cpool.tile([B, 4 * HD], F32)
            nc.sync.dma_start(out=cs_sb[:], in_=cs[:])
            hT_sb = cpool.tile([128, KT, B], BF)
            nc.sync.dma_start(out=hT_sb[:], in_=hT[:])

            # per-request KV loads ([128, pb, HL, 128] each, one contiguous
            # run per partition in DRAM)
            kv_tiles = {}

            def load_b(i):
                b = order[i]
                pb = pairs[b]
                kt_ = kvp.tile([128, pb, HL, 128], BF, tag="k")
                nc.sync.dma_start(out=kt_[:], in_=kc[:, b, 0:pb, :, :])
                vt_ = kvp.tile([128, pb, HL, 128], BF, tag="v")
                nc.sync.dma_start(out=vt_[:], in_=vc[:, b, 0:pb, :, :])
                kv_tiles[i] = (kt_, vt_)

            # accumulators written per-b, read in the epilogue
            atsb = wpool.tile([128, HL * B], F32)   # cached attn, col h*32+b
            nc.vector.memset(atsb[:], 0.0)
            dnm = wpool.tile([1, HL * B], F32)      # cached denom, col h*32+b
            nc.vector.memset(dnm[:], 0.0)

            # o_proj weights: issued right after the wp stream so they fill
            # the DMA pipe while the (serial) rope/transpose phase runs
            wo_tiles = {}

            def issue_wo(i):
                wot = wop.tile([128, HID], BF, tag="wot")
                nc.sync.dma_start(out=wot[:], in_=wo[i])
                wo_tiles[i] = wot

            with tc.tile_pool(name="psA", bufs=1, space="PSUM") as psA:
                # PE warmup transpose so `ident` is observed by PE before the
                # real (fp32, single-wait-slot) transposes below.
                tp0 = psA.tile([B, B], BF, tag="tp0")
                nc.tensor.transpose(tp0[:], ident[:], ident[:])

                # ---- phase 1: qkv = hidden @ W_pack (bf16) ----
                qkv_ps = psA.tile([B, 3 * HD], F32, tag="qkv")
                for kt in range(KT):
                    wpt = wtp.tile([128, 3 * HD], BF, tag="wpt")
                    nc.sync.dma_start(out=wpt[:], in_=wp[kt])
                    for n in range(3):
                        nc.tensor.matmul(
                            qkv_ps[:, n * HD : (n + 1) * HD],
                            hT_sb[:, kt, :],
                            wpt[:, n * HD : (n + 1) * HD],
                            start=(kt == 0),
                            stop=(kt == KT - 1),
                        )

                # KV preloads queue behind the wp stream (the loop can't
                # consume them before phase 2 anyway), then o_proj weights
                for i in range(PRE):
                    if pairs[order[i]] > 0:
                        load_b(i)
                for i in range(HL):
                    issue_wo(i)

                # ---- phase 2: rotary (fp32 DVE, reading PSUM directly),
                # rotated q/k written straight to the bf16 staging tile ----
                qkv_bf = wpool.tile([B, 3 * HD], BF)
                nc.scalar.copy(qkv_bf[:, 2 * HD :], qkv_ps[:, 2 * HD :])

                def rope(src_off, cs_off):
                    src = qkv_ps[:, src_off : src_off + HD]
                    t1 = wpool.tile([B, HD], F32, tag="rope_t1")
                    nc.vector.tensor_tensor(
                        t1[:], src, cs_sb[:, cs_off : cs_off + HD], MUL
                    )
                    sh = wpool.tile([B, HD], F32, tag="rope_sh")
                    sh4 = sh[:].rearrange("b (h d) -> b h d", h=HL)
                    sr4 = src.rearrange("b (h d) -> b h d", h=HL)
                    nc.scalar.copy(sh4[:, :, 0:64], sr4[:, :, 64:128])
                    nc.scalar.copy(sh4[:, :, 64:128], sr4[:, :, 0:64])
                    nc.vector.tensor_tensor(
                        sh[:], sh[:], cs_sb[:, cs_off + HD : cs_off + 2 * HD], MUL
                    )
                    nc.vector.tensor_tensor(
                        qkv_bf[:, src_off : src_off + HD], t1[:], sh[:], ADD
                    )

                rope(0, 0)
                rope(HD, 2 * HD)

            # PE transposes (bf16, permuted to slot order by `ident`)
            qT_bf = wpool.tile([128, HL * B], BF)
            vT = wpool.tile([128, HL * B], F32)
            prod = wpool.tile([128, HL * B], BF)
            with tc.tile_pool(name="psT", bufs=2, space="PSUM") as psT:
                for h in range(HL):
                    tpq = psT.tile([128, B], BF, tag="tpq")
                    nc.tensor.transpose(
                        tpq[:], qkv_bf[:, h * D : (h + 1) * D], ident[:]
                    )
                    tpk = psT.tile([128, B], BF, tag="tpk")
                    nc.tensor.transpose(
                        tpk[:], qkv_bf[:, HD + h * D : HD + (h + 1) * D], ident[:]
                    )
                    tpv = psT.tile([128, B], BF, tag="tpv")
                    nc.tensor.transpose(
                        tpv[:], qkv_bf[:, 2 * HD + h * D : 2 * HD + (h + 1) * D],
                        ident[:],
                    )
                    nc.vector.tensor_copy(qT_bf[:, h * B : (h + 1) * B], tpq[:])
                    nc.scalar.copy(vT[:, h * B : (h + 1) * B], tpv[:])
                    # new-token score terms: q_d * k_d (slot order), bf16
                    nc.vector.tensor_tensor(
                        prod[:, h * B : (h + 1) * B],
                        qT_bf[:, h * B : (h + 1) * B], tpk[:], MUL
                    )

                sn_ps = psT.tile([1, HL * B], F32, tag="sn")
                nc.tensor.matmul(sn_ps[:], ones[:], prod[:], start=True, stop=True)
                e_new = wpool.tile([1, HL * B], F32)
                nc.scalar.activation(e_new[:], sn_ps[:], EXP_FN)

            # ---- phase 3: per-request paged attention (slot order) ----
            # Software-pipelined one request ahead: scores(i+1) is issued
            # before attnV(i) so the PE never stalls on the mask->exp->cast
            # round trip through DVE/ACT.
            ph_tiles = {}
            with (
                tc.tile_pool(name="psB", bufs=3, space="PSUM") as psB,
                tc.tile_pool(name="psB2", bufs=2, space="PSUM") as psB2,
            ):
                def do_scores(i):
                    pb = pairs[order[i]]
                    r = rtail[i]          # valid rows in the last pair
                    kt_, _ = kv_tiles[i]
                    # scores^T: [128(s), (h, pair)]
                    scp = psB.tile([128, HL, pb], F32, tag="scp")
                    for h in range(HL):
                        qh = qT_bf[:, h * B + i : h * B + i + 1]
                        for p in range(pb):
                            nc.tensor.matmul(
                                scp[:, h, p : p + 1], kt_[:, p, h, :], qh,
                                start=True, stop=True,
                            )
                    # probs = exp(scores) in bf16 straight off PSUM; rows
                    # >= hist in the last pair are zeroed by an ACT copy with
                    # a per-partition 0/1 scale (same engine, no extra hop)
                    ph = smp.tile([128, HL, pb], BF, tag="ph")
                    nc.scalar.activation(ph[:], scp[:], EXP_FN)
                    if r < 128:
                        nc.scalar.activation(
                            ph[:, :, pb - 1], ph[:, :, pb - 1], COPY_FN,
                            scale=zmask[:, i : i + 1],
                        )
                    ph_tiles[i] = ph

                def do_attnv(i):
                    pb = pairs[order[i]]
                    _, vt_ = kv_tiles.pop(i)
                    ph = ph_tiles.pop(i)
                    # attn^T[d, h] = sum_s p[s] * V[s, d]
                    atp = psB.tile([128, HL], F32, tag="atp")
                    for h in range(HL):
                        for p in range(pb):
                            nc.tensor.matmul(
                                atp[:, h : h + 1], vt_[:, p, h, :],
                                ph[:, h, p : p + 1],
                                start=(p == 0), stop=(p == pb - 1),
                            )
                    nc.scalar.copy(
                        atsb[:].rearrange("d (h b2) -> d h b2", h=HL)[:, :, i], atp[:]
                    )
                    # denominators: column sums of probs
                    dsp = psB2.tile([1, HL * pb], F32, tag="dsp")
                    nc.tensor.matmul(
                        dsp[:], ones[:],
                        ph[:].rearrange("s h p -> s (h p)"),
                        start=True, stop=True,
                    )
                    nc.vector.reduce_sum(
                        dnm[:].rearrange("o (h b2) -> o h b2", h=HL)[:, :, i],
                        dsp[:].rearrange("o (h p) -> o h p", h=HL),
                        axis=mybir.AxisListType.X,
                    )

                do_scores(0)
                do_scores(1)
                for i in range(B):
                    nxt = i + PRE
                    if nxt < B and nxt not in kv_tiles:
                        load_b(nxt)
                    if i + 2 < B:
                        do_scores(i + 2)
                    do_attnv(i)

            # ---- epilogue: add new token, normalize, project ----
            dtot = wpool.tile([1, HL * B], F32)
            nc.vector.tensor_tensor(dtot[:], dnm[:], e_new[:], ADD)
            rec = wpool.tile([1, HL * B], F32)
            nc.vector.reciprocal(rec[:], dtot[:])
            att = wpool.tile([128, HL * B], F32)
            with tc.tile_pool(name="psD", bufs=1, space="PSUM") as psD:
                # broadcast rows across partitions via K=1 outer products
                ebp = psD.tile([128, HL * B], F32, tag="ebp")
                nc.tensor.matmul(ebp[:], onesf[:], e_new[:], start=True, stop=True)
                rbp = psD.tile([128, HL * B], F32, tag="rbp")
                nc.tensor.matmul(rbp[:], onesf[:], rec[:], start=True, stop=True)

                nc.vector.tensor_tensor(att[:], vT[:], ebp[:], MUL)
                nc.vector.tensor_tensor(att[:], att[:], atsb[:], ADD)
                nc.vector.tensor_tensor(att[:], att[:], rbp[:], MUL)
            at_bf = wpool.tile([128, HL * B], BF)
            nc.vector.tensor_copy(at_bf[:], att[:])

            with tc.tile_pool(name="psC", bufs=3, space="PSUM") as psC:
                outc = wpool.tile([B, HID], F32)
                for n in range(8):
                    opsn = psC.tile([B, 512], F32, tag="ops")
                    for h in range(HL):
                        nc.tensor.matmul(
                            opsn[:],
                            at_bf[:, h * B : (h + 1) * B],
                            wo_tiles[h][:, n * 512 : (n + 1) * 512],
                            start=(h == 0),
                            stop=(h == HL - 1),
                        )
                    oslice = outc[:, n * 512 : (n + 1) * 512]
                    if n % 2:
                        nc.scalar.copy(oslice, opsn[:])
                    else:
                        nc.vector.tensor_copy(oslice, opsn[:])
                nc.sync.dma_start(out=out_part[:], in_=outc[:])

    _split_excess_waits(nc)
    return nc


def _host_prep(hidden, W_pack, o_proj_weight, k_cache, v_cache, hist, block_offsets):
    """Build the 8 per-core input maps (numpy only)."""
    hidden = np.asarray(hidden, np.float32)
    W_pack = np.asarray(W_pack, np.float32)
    o_proj_weight = np.asarray(o_proj_weight, np.float32)
    k_cache = np.asarray(k_cache, np.float32)
    v_cache = np.asarray(v_cache, np.float32)
    hist = np.asarray(hist, np.int64)
    block_offsets = np.asarray(block_offsets, np.int64)

    pairs = [int((h + 127) // 128) for h in hist]
    # slot order: descending pairs so the end of the KV stream (which is no
    # longer overlapped with DMA) carries the cheapest compute
    order = sorted(range(B), key=lambda b: (-pairs[b], b))

    # rope tables, scale folded into the q tables
    inv_freq = 1.0 / (ROPE_BASE ** (np.arange(0, D, 2, dtype=np.float32) / D))
    ang = hist.astype(np.float32)[:, None] * inv_freq[None, :]        # [B, 64]
    cos128 = np.concatenate([np.cos(ang), np.cos(ang)], -1)           # [B, 128]
    sin128 = np.concatenate([np.sin(ang), np.sin(ang)], -1)
    sign = np.concatenate([-np.ones(64), np.ones(64)]).astype(np.float32)
    sc = 1.0 / math.sqrt(D)
    tile_h = lambda x: np.tile(x, (1, HL)).astype(np.float32)         # [B, 512]
    cs = np.concatenate(
        [tile_h(cos128 * sc), tile_h(sin128 * sign * sc),
         tile_h(cos128), tile_h(sin128 * sign)], -1,
    )                                                                 # [B, 2048]

    # valid rows in the last loaded pair, per slot (positions < hist)
    rtail = [int(hist[b]) - 128 * (pairs[b] - 1) for b in order]
    zmask = np.zeros((128, B), dtype=np.float32)
    for i, r in enumerate(rtail):
        zmask[:r, i] = 1.0

    hT = np.ascontiguousarray(hidden.T)                               # [4096, 32]
    hT_bf = np.ascontiguousarray(
        hT.astype(BF_NP).reshape(KT, 128, B).transpose(1, 0, 2)
    )                                                                 # [128, KT, B]

    # gather caches via the block table (b-major), slice heads per core
    k_all = k_cache[block_offsets.reshape(-1)]                        # [512,64,32,128]
    v_all = v_cache[block_offsets.reshape(-1)]

    # permutation matrix: column slot i picks original request order[i]
    ident = np.zeros((B, B), dtype=BF_NP)
    ident[np.asarray(order), np.arange(B)] = 1.0

    in_maps = []
    for c in range(NCORES):
        h0 = c * HL
        qcols = np.arange(h0 * D, (h0 + HL) * D)
        wp_c = np.concatenate(
            [W_pack[:, qcols], W_pack[:, HID + qcols], W_pack[:, 2 * HID + qcols]],
            axis=1,
        )                                                             # [4096, 1536]
        wp_bf = wp_c.astype(BF_NP).reshape(KT, 128, 3 * HL * D)

        wo_c = np.ascontiguousarray(o_proj_weight[:, qcols].T)        # [512, 4096]
        wo_bf = wo_c.astype(BF_NP).reshape(HL, 128, HID)

        kc = k_all[:, :, h0 : h0 + HL, :]                             # [512,64,4,128]
        vc = v_all[:, :, h0 : h0 + HL, :]
        # K: [128(d), B, PAIRS, HL, 128(s)]
        kc5 = kc.reshape(B, PAIRS, 2, BS, HL, D)
        kT_c = np.ascontiguousarray(
            kc5.transpose(5, 0, 1, 4, 2, 3).reshape(D, B, PAIRS, HL, 128)
        ).astype(BF_NP)
        # V: [128(s), B, PAIRS, HL, 128(d)]
        vc5 = vc.reshape(B, PAIRS, 2, BS, HL, D)
        v_c = np.ascontiguousarray(
            vc5.transpose(2, 3, 0, 1, 4, 5).reshape(128, B, PAIRS, HL, D)
        ).astype(BF_NP)

        in_maps.append({
            "hT": hT_bf, "wp": wp_bf, "wo": wo_bf,
            "kc": kT_c, "vc": v_c,
            "cs": cs, "zmask": zmask, "ident": ident,
        })
    return pairs, order, rtail, in_maps


def kernel(hidden_states, W_pack, o_proj_weight, k_cache, v_cache,
           history_lengths, block_offsets):
    global LAST_RESULTS
    pairs, order, rtail, in_maps = _host_prep(
        hidden_states, W_pack, o_proj_weight, k_cache, v_cache,
        history_lengths, block_offsets,
    )
    nc = _build_nc(pairs, order, rtail)
    trace = bool(int(os.environ.get("KERNEL_TRACE", "0")))
    res = run_bass_kernel_spmd(nc, in_maps, list(range(NCORES)), trace=trace)
    LAST_RESULTS = res
    acc = np.zeros((B, HID), np.float32)
    for c in range(NCORES):
        acc += res.results[c]["out_part"]
    out = np.zeros((B, HID), np.float32)
    out[np.asarray(order)] = acc                   # slot rows -> original rows
    return out


# revision 29
# speedup vs baseline: 1.0482x; 1.0150x over previous
"""Paged decode attention (nn_Attention_5626407157951) on 8 Trainium2 cores.

Tensor-parallel over heads: each core owns 4 of 32 heads. Per core:
  qkv = hidden @ W_pack[:, own cols]      (bf16 matmuls, fp32 acc)
  rotary(q, k) at pos=hist                (DVE, fp32; host-built cos/sin)
  scores_T[s, (h,pair)] = K_pair^T q      (PE, K stationary bf16, q moving)
  softmax without max-subtraction; new token handled analytically:
      out = (sum_s exp(s)*v_s + e_new*v_new) / (sum_s exp(s) + e_new)
  out_partial = attn @ o_proj[:, own dims].T ; host sums the 8 partials.

Everything is bf16 (2 bytes/elem): the correctness gate is rel_err < 2e-2
and pure-bf16 lands ~7e-3, so no hi/lo error-compensation splits are
needed. This halves HBM traffic vs a 3-byte hi/lo scheme and cuts the
matmul count 3x. Host pre-transposes weights/caches into DMA-friendly
layouts with large contiguous runs per partition.
"""

import math
import os

import ml_dtypes
import numpy as np

import concourse.bass as bass
import concourse.mybir as mybir
import concourse.tile as tile
from concourse.bass_utils import run_bass_kernel_spmd
from concourse.vector_clock import ScopedClock

B = 32          # batch (decode requests)
H = 32          # total heads
HL = 4          # heads per core
D = 128         # head dim
HID = 4096
BS = 64         # cache block size
NBLK = 16       # blocks per request
NCORES = 8
KT = HID // 128         # 32 contraction tiles for qkv proj
PAIRS = NBLK // 2       # 8 block-pairs (128 tokens each) per request
ROPE_BASE = 10000.0
PRE = 5                 # KV prefetch depth (requests ahead)

F32 = mybir.dt.float32
BF = mybir.dt.bfloat16
BF_NP = ml_dtypes.bfloat16
EXP_FN = mybir.ActivationFunctionType.Exp
COPY_FN = mybir.ActivationFunctionType.Copy
MUL = mybir.AluOpType.mult
ADD = mybir.AluOpType.add
SUB = mybir.AluOpType.subtract
DIV = mybir.AluOpType.divide

LAST_RESULTS = None  # test harness peeks at this for profiling info

# ---------------------------------------------------------------------------
# This walrus build accepts very few sync-waits per instruction; the Tile
# kernel-tail drain accumulates one wait per sem lane. Split the waits over
# several drain instructions (all before the barrier, so semantics hold).
_MAX_DRAIN_WAITS = 1


def _patched_drain_and_barrier(self, tick_clock, wait_clock):
    nc = self.nc
    drain_inst = nc.sync.drain()
    wait_clock.add_sem_waits(
        drain_inst.ins, ScopedClock({None: tick_clock.global_clock})
    )
    si = drain_inst.ins.sync_info
    if si is not None and si.on_wait and len(si.on_wait) > _MAX_DRAIN_WAITS:
        waits = list(si.on_wait)
        drain_inst.ins.sync_info = mybir.SyncInfo(
            on_wait=waits[:_MAX_DRAIN_WAITS], on_update=list(si.on_update or [])
        )
        rest = waits[_MAX_DRAIN_WAITS:]
        for i in range(0, len(rest), _MAX_DRAIN_WAITS):
            extra = nc.sync.drain()
            extra.ins.sync_info = mybir.SyncInfo(
                on_wait=rest[i : i + _MAX_DRAIN_WAITS], on_update=[]
            )
    nc.all_engine_barrier()
    popped = nc._tile_sem_poison_stack.pop()
    assert popped is self._sem_poison
    nc.clear_and_free_semaphores(list(self.sems.allocated().values()))
    nc.all_engine_barrier()


tile.TileContext._drain_and_barrier = _patched_drain_and_barrier


def _split_excess_waits(nc, limit=1):
    """Walrus rejects instructions carrying more than ~1 sync wait. Hoist the
    excess onto NoOps inserted just before, on the same engine queue (the
    queue blocks on them first, so semantics are identical)."""
    for fn in nc.m.functions:
        for bb in fn.blocks:
            out = []
            changed = False
            for inst in list(bb.instructions):
                si = getattr(inst, "sync_info", None)
                if si is not None and si.on_wait and len(si.on_wait) > limit:
                    waits = list(si.on_wait)
                    extra, keep = waits[:-limit], waits[-limit:]
                    for i in range(0, len(extra), limit):
                        nop = mybir.InstNoOp(
                            name=nc.get_next_instruction_name(),
                            ins=[], outs=[], engine=inst.engine,
                            sync_info=mybir.SyncInfo(
                                on_wait=extra[i : i + limit], on_update=[]
                            ),
                        )
                        nc.register_instruction(nop)
                        out.append(nop)
                    inst.sync_info = mybir.SyncInfo(
                        on_wait=keep, on_update=list(si.on_update or [])
                    )
                    changed = True
                out.append(inst)
            if changed:
                bb.instructions = out
# ---------------------------------------------------------------------------


def _build_nc(pairs, order, rtail):
    """Build the SPMD bass module. `pairs[b]` = number of 128-token cached
    pairs for request b (same on every core; head split is via input data).
    `order[i]` = original request processed in slot i (descending pairs, so
    the tail of the stream is the cheapest compute). The `ident` input is a
    permutation matrix mapping qkv rows (original b) to slot columns; the
    host unpermutes the output rows."""
    nc = bass.Bass()

    def param(name, shape, dt):
        return nc.declare_dram_parameter(name, list(shape), dt, isOutput=False)

    hT = param("hT", [128, KT, B], BF)
    wp = param("wp", [KT, 128, 3 * HL * D], BF)
    wo = param("wo", [HL, 128, HID], BF)
    kc = param("kc", [128, B, PAIRS, HL, 128], BF)   # [d, b, pair, h, s]
    vc = param("vc", [128, B, PAIRS, HL, 128], BF)   # [s, b, pair, h, d]
    cs = param("cs", [B, 4 * HL * D], F32)
    zmaskp = param("zmask", [128, B], F32)            # 1 iff row s < hist (last pair)
    identp = param("ident", [B, B], BF)              # permutation matrix
    out_part = nc.declare_dram_parameter("out_part", [B, HID], F32, isOutput=True)

    HD = HL * D  # 512 local attention dims

    with tile.TileContext(nc) as tc:
        with (
            tc.tile_pool(name="const", bufs=1) as cpool,
            tc.tile_pool(name="work", bufs=1) as wpool,
            tc.tile_pool(name="wtiles", bufs=12) as wtp,
            tc.tile_pool(name="wop", bufs=4) as wop,
            tc.tile_pool(name="kv", bufs=PRE + 1) as kvp,
            tc.tile_pool(name="small", bufs=3) as smp,
        ):
            # ---- constants ----
            ident = cpool.tile([B, B], BF)
            nc.sync.dma_start(out=ident[:], in_=identp[:])
            ones = cpool.tile([128, 1], BF)
            nc.vector.memset(ones[:], 1.0)
            onesb = cpool.tile([1, HL * B], BF)
            nc.vector.memset(onesb[:], 1.0)
            zmask = cpool.tile([128, B], F32)
            nc.sync.dma_start(out=zmask[:], in_=zmaskp[:])
            cs_sb = cpool.tile([B, 4 * HD], F32)
            nc.sync.dma_start(out=cs_sb[:], in_=cs[:])
            hT_sb = cpool.tile([128, KT, B], BF)
            nc.sync.dma_start(out=hT_sb[:], in_=hT[:])

            # per-request KV loads ([128, pb, HL, 128] each, one contiguous
            # run per partition in DRAM)
            kv_tiles = {}

            def load_b(i):
                b = order[i]
                pb = pairs[b]
                kt_ = kvp.tile([128, pb, HL, 128], BF, tag="k")
                nc.sync.dma_start(out=kt_[:], in_=kc[:, b, 0:pb, :, :])
                vt_ = kvp.tile([128, pb, HL, 128], BF, tag="v")
                nc.sync.dma_start(out=vt_[:], in_=vc[:, b, 0:pb, :, :])
                kv_tiles[i] = (kt_, vt_)

            # accumulators written per-b, read in the epilogue
            atsb = wpool.tile([128, HL * B], F32)   # cached attn, col h*32+b
            nc.vector.memset(atsb[:], 0.0)
            dnm = wpool.tile([1, HL * B], F32)      # cached denom, col h*32+b
            nc.vector.memset(dnm[:], 0.0)

            # o_proj weights: issued right after the wp stream so they fill
            # the DMA pipe while the (serial) rope/transpose phase runs
            wo_tiles = {}

            def issue_wo(i):
                wot = wop.tile([128, HID], BF, tag="wot")
                nc.sync.dma_start(out=wot[:], in_=wo[i])
                wo_tiles[i] = wot

            with tc.tile_pool(name="psA", bufs=1, space="PSUM") as psA:
                # PE warmup transpose so `ident` is observed by PE before the
                # real (fp32, single-wait-slot) transposes below.
                tp0 = psA.tile([B, B], BF, tag="tp0")
                nc.tensor.transpose(tp0[:], ident[:], ident[:])

                # ---- phase 1: qkv = hidden @ W_pack (bf16) ----
                qkv_ps = psA.tile([B, 3 * HD], F32, tag="qkv")
                for kt in range(KT):
                    wpt = wtp.tile([128, 3 * HD], BF, tag="wpt")
                    nc.sync.dma_start(out=wpt[:], in_=wp[kt])
                    for n in range(3):
                        nc.tensor.matmul(
                            qkv_ps[:, n * HD : (n + 1) * HD],
                            hT_sb[:, kt, :],
                            wpt[:, n * HD : (n + 1) * HD],
                            start=(kt == 0),
                            stop=(kt == KT - 1),
                        )

                # KV preloads queue behind the wp stream (the loop can't
                # consume them before phase 2 anyway)
                for i in range(PRE):
                    if pairs[order[i]] > 0:
                        load_b(i)

                # ---- phase 2: rotary (fp32 DVE, reading PSUM directly),
                # rotated q/k written straight to the bf16 staging tile ----
                qkv_bf = wpool.tile([B, 3 * HD], BF)
                nc.scalar.copy(qkv_bf[:, 2 * HD :], qkv_ps[:, 2 * HD :])

                def rope(src_off, cs_off):
                    src = qkv_ps[:, src_off : src_off + HD]
                    t1 = wpool.tile([B, HD], F32, tag="rope_t1")
                    nc.vector.tensor_tensor(
                        t1[:], src, cs_sb[:, cs_off : cs_off + HD], MUL
                    )
                    sh = wpool.tile([B, HD], F32, tag="rope_sh")
                    sh4 = sh[:].rearrange("b (h d) -> b h d", h=HL)
                    sr4 = src.rearrange("b (h d) -> b h d", h=HL)
                    nc.scalar.copy(sh4[:, :, 0:64], sr4[:, :, 64:128])
                    nc.scalar.copy(sh4[:, :, 64:128], sr4[:, :, 0:64])
                    nc.vector.tensor_tensor(
                        sh[:], sh[:], cs_sb[:, cs_off + HD : cs_off + 2 * HD], MUL
                    )
                    nc.vector.tensor_tensor(
                        qkv_bf[:, src_off : src_off + HD], t1[:], sh[:], ADD
                    )

                rope(0, 0)
                rope(HD, 2 * HD)

            # PE transposes (bf16, permuted to slot order by `ident`)
            qT_bf = wpool.tile([128, HL * B], BF)
            vT = wpool.tile([128, HL * B], F32)
            prod = wpool.tile([128, HL * B], BF)
            with tc.tile_pool(name="psT", bufs=2, space="PSUM") as psT:
                for h in range(HL):
                    tpq = psT.tile([128, B], BF, tag="tpq")
                    nc.tensor.transpose(
                        tpq[:], qkv_bf[:, h * D : (h + 1) * D], ident[:]
                    )
                    tpk = psT.tile([128, B], BF, tag="tpk")
                    nc.tensor.transpose(
                        tpk[:], qkv_bf[:, HD + h * D : HD + (h + 1) * D], ident[:]
                    )
                    tpv = psT.tile([128, B], BF, tag="tpv")
                    nc.tensor.transpose(
                        tpv[:], qkv_bf[:, 2 * HD + h * D : 2 * HD + (h + 1) * D],
                        ident[:],
                    )
                    nc.vector.tensor_copy(qT_bf[:, h * B : (h + 1) * B], tpq[:])
                    nc.scalar.copy(vT[:, h * B : (h + 1) * B], tpv[:])
                    # new-token score terms: q_d * k_d (slot order), bf16
                    nc.vector.tensor_tensor(
                        prod[:, h * B : (h + 1) * B],
                        qT_bf[:, h * B : (h + 1) * B], tpk[:], MUL
                    )

                sn_ps = psT.tile([1, HL * B], F32, tag="sn")
                nc.tensor.matmul(sn_ps[:], ones[:], prod[:], start=True, stop=True)
                e_new = wpool.tile([1, HL * B], F32)
                nc.scalar.activation(e_new[:], sn_ps[:], EXP_FN)

            dtot = wpool.tile([1, HL * B], F32)
            rec = wpool.tile([1, HL * B], F32)

            # ---- phase 3: per-request paged attention (slot order) ----
            # Software-pipelined one request ahead: scores(i+1) is issued
            # before attnV(i) so the PE never stalls on the mask->exp->cast
            # round trip through DVE/ACT.
            ph_tiles = {}
            with (
                tc.tile_pool(name="psB", bufs=3, space="PSUM") as psB,
                tc.tile_pool(name="psB2", bufs=2, space="PSUM") as psB2,
            ):
                def do_scores(i):
                    pb = pairs[order[i]]
                    r = rtail[i]          # valid rows in the last pair
                    kt_, _ = kv_tiles[i]
                    # scores^T: [128(s), (h, pair)]
                    scp = psB.tile([128, HL, pb], F32, tag="scp")
                    for h in range(HL):
                        qh = qT_bf[:, h * B + i : h * B + i + 1]
                        for p in range(pb):
                            nc.tensor.matmul(
                                scp[:, h, p : p + 1], kt_[:, p, h, :], qh,
                                start=True, stop=True,
                            )
                    # probs = exp(scores) in bf16 straight off PSUM; rows
                    # >= hist in the last pair are zeroed by an ACT copy with
                    # a per-partition 0/1 scale (same engine, no extra hop)
                    ph = smp.tile([128, HL, pb], BF, tag="ph")
                    nc.scalar.activation(ph[:], scp[:], EXP_FN)
                    if r < 128:
                        nc.scalar.activation(
                            ph[:, :, pb - 1], ph[:, :, pb - 1], COPY_FN,
                            scale=zmask[:, i : i + 1],
                        )
                    ph_tiles[i] = ph

                def do_attnv(i):
                    pb = pairs[order[i]]
                    _, vt_ = kv_tiles.pop(i)
                    ph = ph_tiles.pop(i)
                    # attn^T[d, h] = sum_s p[s] * V[s, d]
                    atp = psB.tile([128, HL], F32, tag="atp")
                    for h in range(HL):
                        for p in range(pb):
                            nc.tensor.matmul(
                                atp[:, h : h + 1], vt_[:, p, h, :],
                                ph[:, h, p : p + 1],
                                start=(p == 0), stop=(p == pb - 1),
                            )
                    nc.scalar.copy(
                        atsb[:].rearrange("d (h b2) -> d h b2", h=HL)[:, :, i], atp[:]
                    )
                    # denominators: column sums of probs
                    dsp = psB2.tile([1, HL * pb], F32, tag="dsp")
                    nc.tensor.matmul(
                        dsp[:], ones[:],
                        ph[:].rearrange("s h p -> s (h p)"),
                        start=True, stop=True,
                    )
                    nc.vector.reduce_sum(
                        dnm[:].rearrange("o (h b2) -> o h b2", h=HL)[:, :, i],
                        dsp[:].rearrange("o (h p) -> o h p", h=HL),
                        axis=mybir.AxisListType.X,
                    )
                    # incremental 1/(denom + e_new) for this slot (tiny, off
                    # the critical tail)
                    dslc = dnm[:].rearrange("o (h b2) -> o h b2", h=HL)[:, :, i]
                    eslc = e_new[:].rearrange("o (h b2) -> o h b2", h=HL)[:, :, i]
                    tslc = dtot[:].rearrange("o (h b2) -> o h b2", h=HL)[:, :, i]
                    rslc = rec[:].rearrange("o (h b2) -> o h b2", h=HL)[:, :, i]
                    nc.vector.tensor_tensor(tslc, dslc, eslc, ADD)
                    nc.vector.reciprocal(rslc, tslc)

                # o_proj weights slot into the KV stream mid-loop; they
                # are only needed at the very end
                wo_sched = {6: 0, 10: 1, 14: 2, 18: 3}
                do_scores(0)
                do_scores(1)
                for i in range(B):
                    if i in wo_sched:
                        issue_wo(wo_sched[i])
                    nxt = i + PRE
                    if nxt < B and nxt not in kv_tiles:
                        load_b(nxt)
                    if i + 2 < B:
                        do_scores(i + 2)
                    do_attnv(i)

            # ---- epilogue: add new token, normalize, project ----
            e_bf = wpool.tile([1, HL * B], BF)
            nc.scalar.copy(e_bf[:], e_new[:])
            r_bf = wpool.tile([1, HL * B], BF)
            nc.scalar.copy(r_bf[:], rec[:])
            att = wpool.tile([128, HL * B], F32)
            with tc.tile_pool(name="psD", bufs=1, space="PSUM") as psD:
                # broadcast rows across partitions via K=1 outer products
                ebp = psD.tile([128, HL * B], F32, tag="ebp")
                nc.tensor.matmul(ebp[:], onesb[:], e_bf[:], start=True, stop=True)
                rbp = psD.tile([128, HL * B], F32, tag="rbp")
                nc.tensor.matmul(rbp[:], onesb[:], r_bf[:], start=True, stop=True)

                nc.vector.tensor_tensor(att[:], vT[:], ebp[:], MUL)
                nc.vector.tensor_tensor(att[:], att[:], atsb[:], ADD)
                nc.vector.tensor_tensor(att[:], att[:], rbp[:], MUL)
            at_bf = wpool.tile([128, HL * B], BF)
            nc.vector.tensor_copy(at_bf[:], att[:])

            with tc.tile_pool(name="psC", bufs=3, space="PSUM") as psC:
                for n in range(8):
                    opsn = psC.tile([B, 512], F32, tag="ops")
                    for h in range(HL):
                        nc.tensor.matmul(
                            opsn[:],
                            at_bf[:, h * B : (h + 1) * B],
                            wo_tiles[h][:, n * 512 : (n + 1) * 512],
                            start=(h == 0),
                            stop=(h == HL - 1),
                        )
                    outc = smp.tile([B, 512], F32, tag="outc")
                    if n % 2:
                        nc.scalar.copy(outc[:], opsn[:])
                    else:
                        nc.vector.tensor_copy(outc[:], opsn[:])
                    nc.sync.dma_start(
                        out=out_part[:, n * 512 : (n + 1) * 512], in_=outc[:]
                    )

    _split_excess_waits(nc)
    return nc


def _host_prep(hidden, W_pack, o_proj_weight, k_cache, v_cache, hist, block_offsets):
    """Build the 8 per-core input maps (numpy only)."""
    hidden = np.asarray(hidden, np.float32)
    W_pack = np.asarray(W_pack, np.float32)
    o_proj_weight = np.asarray(o_proj_weight, np.float32)
    k_cache = np.asarray(k_cache, np.float32)
    v_cache = np.asarray(v_cache, np.float32)
    hist = np.asarray(hist, np.int64)
    block_offsets = np.asarray(block_offsets, np.int64)

    pairs = [int((h + 127) // 128) for h in hist]
    # slot order: descending pairs so the end of the KV stream (which is no
    # longer overlapped with DMA) carries the cheapest compute
    order = sorted(range(B), key=lambda b: (-pairs[b], b))

    # rope tables, scale folded into the q tables
    inv_freq = 1.0 / (ROPE_BASE ** (np.arange(0, D, 2, dtype=np.float32) / D))
    ang = hist.astype(np.float32)[:, None] * inv_freq[None, :]        # [B, 64]
    cos128 = np.concatenate([np.cos(ang), np.cos(ang)], -1)           # [B, 128]
    sin128 = np.concatenate([np.sin(ang), np.sin(ang)], -1)
    sign = np.concatenate([-np.ones(64), np.ones(64)]).astype(np.float32)
    sc = 1.0 / math.sqrt(D)
    tile_h = lambda x: np.tile(x, (1, HL)).astype(np.float32)         # [B, 512]
    cs = np.concatenate(
        [tile_h(cos128 * sc), tile_h(sin128 * sign * sc),
         tile_h(cos128), tile_h(sin128 * sign)], -1,
    )                                                                 # [B, 2048]

    # valid rows in the last loaded pair, per slot (positions < hist)
    rtail = [int(hist[b]) - 128 * (pairs[b] - 1) for b in order]
    zmask = np.zeros((128, B), dtype=np.float32)
    for i, r in enumerate(rtail):
        zmask[:r, i] = 1.0

    hT = np.ascontiguousarray(hidden.T)                               # [4096, 32]
    hT_bf = np.ascontiguousarray(
        hT.astype(BF_NP).reshape(KT, 128, B).transpose(1, 0, 2)
    )                                                                 # [128, KT, B]

    # gather caches via the block table (b-major), slice heads per core
    k_all = k_cache[block_offsets.reshape(-1)]                        # [512,64,32,128]
    v_all = v_cache[block_offsets.reshape(-1)]

    # permutation matrix: column slot i picks original request order[i]
    ident = np.zeros((B, B), dtype=BF_NP)
    ident[np.asarray(order), np.arange(B)] = 1.0

    in_maps = []
    for c in range(NCORES):
        h0 = c * HL
        qcols = np.arange(h0 * D, (h0 + HL) * D)
        wp_c = np.concatenate(
            [W_pack[:, qcols], W_pack[:, HID + qcols], W_pack[:, 2 * HID + qcols]],
            axis=1,
        )                                                             # [4096, 1536]
        wp_bf = wp_c.astype(BF_NP).reshape(KT, 128, 3 * HL * D)

        wo_c = np.ascontiguousarray(o_proj_weight[:, qcols].T)        # [512, 4096]
        wo_bf = wo_c.astype(BF_NP).reshape(HL, 128, HID)

        kc = k_all[:, :, h0 : h0 + HL, :]                             # [512,64,4,128]
        vc = v_all[:, :, h0 : h0 + HL, :]
        # K: [128(d), B, PAIRS, HL, 128(s)]
        kc5 = kc.reshape(B, PAIRS, 2, BS, HL, D)
        kT_c = np.ascontiguousarray(
            kc5.transpose(5, 0, 1, 4, 2, 3).reshape(D, B, PAIRS, HL, 128)
        ).astype(BF_NP)
        # V: [128(s), B, PAIRS, HL, 128(d)]
        vc5 = vc.reshape(B, PAIRS, 2, BS, HL, D)
        v_c = np.ascontiguousarray(
            vc5.transpose(2, 3, 0, 1, 4, 5).reshape(128, B, PAIRS, HL, D)
        ).astype(BF_NP)

        in_maps.append({
            "hT": hT_bf, "wp": wp_bf, "wo": wo_bf,
            "kc": kT_c, "vc": v_c,
            "cs": cs, "zmask": zmask, "ident": ident,
        })
    return pairs, order, rtail, in_maps


def kernel(hidden_states, W_pack, o_proj_weight, k_cache, v_cache,
           history_lengths, block_offsets):
    global LAST_RESULTS
    pairs, order, rtail, in_maps = _host_prep(
        hidden_states, W_pack, o_proj_weight, k_cache, v_cache,
        history_lengths, block_offsets,
    )
    nc = _build_nc(pairs, order, rtail)
    trace = bool(int(os.environ.get("KERNEL_TRACE", "0")))
    res = run_bass_kernel_spmd(nc, in_maps, list(range(NCORES)), trace=trace)
    LAST_RESULTS = res
    acc = np.zeros((B, HID), np.float32)
    for c in range(NCORES):
        acc += res.results[c]["out_part"]
    out = np.zeros((B, HID), np.float32)
    out[np.asarray(order)] = acc                   # slot rows -> original rows
    return out


# revision 30
# speedup vs baseline: 1.1182x; 1.0668x over previous
"""Paged decode attention (nn_Attention_5626407157951) on 8 Trainium2 cores.

Tensor-parallel over heads: each core owns 4 of 32 heads. Per core:
  qkv = hidden @ W_pack[:, own cols]      (bf16 matmuls, fp32 acc)
  rotary(q, k) at pos=hist                (DVE, fp32; host-built cos/sin)
  scores_T[s, (h,pair)] = K_pair^T q      (PE, K stationary bf16, q moving)
  softmax without max-subtraction; new token handled analytically:
      out = (sum_s exp(s)*v_s + e_new*v_new) / (sum_s exp(s) + e_new)
  out_partial = attn @ o_proj[:, own dims].T ; host sums the 8 partials.

Everything is bf16 (2 bytes/elem): the correctness gate is rel_err < 2e-2
and pure-bf16 lands ~7e-3, so no hi/lo error-compensation splits are
needed. This halves HBM traffic vs a 3-byte hi/lo scheme and cuts the
matmul count 3x. Host pre-transposes weights/caches into DMA-friendly
layouts with large contiguous runs per partition.
"""

import math
import os

import ml_dtypes
import numpy as np

import concourse.bass as bass
import concourse.mybir as mybir
import concourse.tile as tile
from concourse.bass_utils import run_bass_kernel_spmd
from concourse.vector_clock import ScopedClock

B = 32          # batch (decode requests)
H = 32          # total heads
HL = 4          # heads per core
D = 128         # head dim
HID = 4096
BS = 64         # cache block size
NBLK = 16       # blocks per request
NCORES = 8
KT = HID // 128         # 32 contraction tiles for qkv proj
PAIRS = NBLK // 2       # 8 block-pairs (128 tokens each) per request
ROPE_BASE = 10000.0
PRE = 5                 # KV prefetch depth (requests ahead)

F32 = mybir.dt.float32
BF = mybir.dt.bfloat16
BF_NP = ml_dtypes.bfloat16
EXP_FN = mybir.ActivationFunctionType.Exp
COPY_FN = mybir.ActivationFunctionType.Copy
MUL = mybir.AluOpType.mult
ADD = mybir.AluOpType.add
SUB = mybir.AluOpType.subtract
DIV = mybir.AluOpType.divide

LAST_RESULTS = None  # test harness peeks at this for profiling info

# ---------------------------------------------------------------------------
# This walrus build accepts very few sync-waits per instruction; the Tile
# kernel-tail drain accumulates one wait per sem lane. Split the waits over
# several drain instructions (all before the barrier, so semantics hold).
_MAX_DRAIN_WAITS = 1


def _patched_drain_and_barrier(self, tick_clock, wait_clock):
    nc = self.nc
    drain_inst = nc.sync.drain()
    wait_clock.add_sem_waits(
        drain_inst.ins, ScopedClock({None: tick_clock.global_clock})
    )
    si = drain_inst.ins.sync_info
    if si is not None and si.on_wait and len(si.on_wait) > _MAX_DRAIN_WAITS:
        waits = list(si.on_wait)
        drain_inst.ins.sync_info = mybir.SyncInfo(
            on_wait=waits[:_MAX_DRAIN_WAITS], on_update=list(si.on_update or [])
        )
        rest = waits[_MAX_DRAIN_WAITS:]
        for i in range(0, len(rest), _MAX_DRAIN_WAITS):
            extra = nc.sync.drain()
            extra.ins.sync_info = mybir.SyncInfo(
                on_wait=rest[i : i + _MAX_DRAIN_WAITS], on_update=[]
            )
    nc.all_engine_barrier()
    popped = nc._tile_sem_poison_stack.pop()
    assert popped is self._sem_poison
    nc.clear_and_free_semaphores(list(self.sems.allocated().values()))
    nc.all_engine_barrier()


tile.TileContext._drain_and_barrier = _patched_drain_and_barrier


def _split_excess_waits(nc, limit=1):
    """Walrus rejects instructions carrying more than ~1 sync wait. Hoist the
    excess onto NoOps inserted just before, on the same engine queue (the
    queue blocks on them first, so semantics are identical)."""
    for fn in nc.m.functions:
        for bb in fn.blocks:
            out = []
            changed = False
            for inst in list(bb.instructions):
                si = getattr(inst, "sync_info", None)
                if si is not None and si.on_wait and len(si.on_wait) > limit:
                    waits = list(si.on_wait)
                    extra, keep = waits[:-limit], waits[-limit:]
                    for i in range(0, len(extra), limit):
                        nop = mybir.InstNoOp(
                            name=nc.get_next_instruction_name(),
                            ins=[], outs=[], engine=inst.engine,
                            sync_info=mybir.SyncInfo(
                                on_wait=extra[i : i + limit], on_update=[]
                            ),
                        )
                        nc.register_instruction(nop)
                        out.append(nop)
                    inst.sync_info = mybir.SyncInfo(
                        on_wait=keep, on_update=list(si.on_update or [])
                    )
                    changed = True
                out.append(inst)
            if changed:
                bb.instructions = out
# ---------------------------------------------------------------------------


def _build_nc(pairs, order, rtail):
    """Build the SPMD bass module. `pairs[b]` = number of 128-token cached
    pairs for request b (same on every core; head split is via input data).
    `order[i]` = original request processed in slot i (descending pairs, so
    the tail of the stream is the cheapest compute). The `ident` input is a
    permutation matrix mapping qkv rows (original b) to slot columns; the
    host unpermutes the output rows."""
    nc = bass.Bass()

    def param(name, shape, dt):
        return nc.declare_dram_parameter(name, list(shape), dt, isOutput=False)

    hT = param("hT", [128, KT, B], BF)
    wp = param("wp", [KT, 128, 3 * HL * D], BF)
    wo = param("wo", [HL, 128, HID], BF)
    # K (d-major) and V (s-major) interleaved so each request is ONE DMA
    kvc = param("kvc", [128, B, PAIRS, HL, 2, 128], BF)
    cs = param("cs", [B, 4 * HL * D], F32)
    zmaskp = param("zmask", [128, B], F32)            # 1 iff row s < hist (last pair)
    identp = param("ident", [B, B], BF)              # permutation matrix
    out_part = nc.declare_dram_parameter("out_part", [B, HID], F32, isOutput=True)

    HD = HL * D  # 512 local attention dims

    with tile.TileContext(nc) as tc:
        with (
            tc.tile_pool(name="const", bufs=1) as cpool,
            tc.tile_pool(name="work", bufs=1) as wpool,
            tc.tile_pool(name="wtiles", bufs=12) as wtp,
            tc.tile_pool(name="wop", bufs=4) as wop,
            tc.tile_pool(name="kv", bufs=PRE + 1) as kvp,
            tc.tile_pool(name="small", bufs=3) as smp,
        ):
            # ---- constants ----
            ident = cpool.tile([B, B], BF)
            nc.sync.dma_start(out=ident[:], in_=identp[:])
            ones = cpool.tile([128, 1], BF)
            nc.vector.memset(ones[:], 1.0)
            onesb = cpool.tile([1, HL * B], BF)
            nc.vector.memset(onesb[:], 1.0)
            zmask = cpool.tile([128, B], F32)
            nc.sync.dma_start(out=zmask[:], in_=zmaskp[:])
            cs_sb = cpool.tile([B, 4 * HD], F32)
            nc.sync.dma_start(out=cs_sb[:], in_=cs[:])
            hT_sb = cpool.tile([128, KT, B], BF)
            nc.sync.dma_start(out=hT_sb[:], in_=hT[:])

            # per-request KV loads ([128, pb, HL, 128] each, one contiguous
            # run per partition in DRAM)
            kv_tiles = {}

            def load_b(i):
                b = order[i]
                pb = pairs[b]
                kvt = kvp.tile([128, pb, HL, 2, 128], BF, tag="kv")
                nc.sync.dma_start(out=kvt[:], in_=kvc[:, b, 0:pb, :, :, :])
                kv_tiles[i] = kvt

            # accumulators written per-b, read in the epilogue
            atsb = wpool.tile([128, HL * B], F32)   # cached attn, col h*32+b
            nc.vector.memset(atsb[:], 0.0)
            dnm = wpool.tile([1, HL * B], F32)      # cached denom, col h*32+b
            nc.vector.memset(dnm[:], 0.0)

            # o_proj weights: issued right after the wp stream so they fill
            # the DMA pipe while the (serial) rope/transpose phase runs
            wo_tiles = {}

            def issue_wo(i):
                wot = wop.tile([128, HID], BF, tag="wot")
                nc.sync.dma_start(out=wot[:], in_=wo[i])
                wo_tiles[i] = wot

            with tc.tile_pool(name="psA", bufs=1, space="PSUM") as psA:
                # PE warmup transpose so `ident` is observed by PE before the
                # real (fp32, single-wait-slot) transposes below.
                tp0 = psA.tile([B, B], BF, tag="tp0")
                nc.tensor.transpose(tp0[:], ident[:], ident[:])

                # ---- phase 1: qkv = hidden @ W_pack (bf16) ----
                qkv_ps = psA.tile([B, 3 * HD], F32, tag="qkv")
                for kt in range(KT):
                    wpt = wtp.tile([128, 3 * HD], BF, tag="wpt")
                    nc.sync.dma_start(out=wpt[:], in_=wp[kt])
                    for n in range(3):
                        nc.tensor.matmul(
                            qkv_ps[:, n * HD : (n + 1) * HD],
                            hT_sb[:, kt, :],
                            wpt[:, n * HD : (n + 1) * HD],
                            start=(kt == 0),
                            stop=(kt == KT - 1),
                        )

                # KV preloads queue behind the wp stream (the loop can't
                # consume them before phase 2 anyway)
                for i in range(PRE):
                    if pairs[order[i]] > 0:
                        load_b(i)

                # ---- phase 2: rotary (fp32 DVE, reading PSUM directly),
                # rotated q/k written straight to the bf16 staging tile ----
                qkv_bf = wpool.tile([B, 3 * HD], BF)
                nc.scalar.copy(qkv_bf[:, 2 * HD :], qkv_ps[:, 2 * HD :])

                def rope(src_off, cs_off):
                    src = qkv_ps[:, src_off : src_off + HD]
                    t1 = wpool.tile([B, HD], F32, tag="rope_t1")
                    nc.vector.tensor_tensor(
                        t1[:], src, cs_sb[:, cs_off : cs_off + HD], MUL
                    )
                    sh = wpool.tile([B, HD], F32, tag="rope_sh")
                    sh4 = sh[:].rearrange("b (h d) -> b h d", h=HL)
                    sr4 = src.rearrange("b (h d) -> b h d", h=HL)
                    nc.scalar.copy(sh4[:, :, 0:64], sr4[:, :, 64:128])
                    nc.scalar.copy(sh4[:, :, 64:128], sr4[:, :, 0:64])
                    nc.vector.tensor_tensor(
                        sh[:], sh[:], cs_sb[:, cs_off + HD : cs_off + 2 * HD], MUL
                    )
                    nc.vector.tensor_tensor(
                        qkv_bf[:, src_off : src_off + HD], t1[:], sh[:], ADD
                    )

                rope(0, 0)
                rope(HD, 2 * HD)

            # PE transposes (bf16, permuted to slot order by `ident`)
            qT_bf = wpool.tile([128, HL * B], BF)
            vT = wpool.tile([128, HL * B], F32)
            prod = wpool.tile([128, HL * B], BF)
            with tc.tile_pool(name="psT", bufs=2, space="PSUM") as psT:
                for h in range(HL):
                    tpq = psT.tile([128, B], BF, tag="tpq")
                    nc.tensor.transpose(
                        tpq[:], qkv_bf[:, h * D : (h + 1) * D], ident[:]
                    )
                    tpk = psT.tile([128, B], BF, tag="tpk")
                    nc.tensor.transpose(
                        tpk[:], qkv_bf[:, HD + h * D : HD + (h + 1) * D], ident[:]
                    )
                    tpv = psT.tile([128, B], BF, tag="tpv")
                    nc.tensor.transpose(
                        tpv[:], qkv_bf[:, 2 * HD + h * D : 2 * HD + (h + 1) * D],
                        ident[:],
                    )
                    nc.vector.tensor_copy(qT_bf[:, h * B : (h + 1) * B], tpq[:])
                    nc.scalar.copy(vT[:, h * B : (h + 1) * B], tpv[:])
                    # new-token score terms: q_d * k_d (slot order), bf16
                    nc.vector.tensor_tensor(
                        prod[:, h * B : (h + 1) * B],
                        qT_bf[:, h * B : (h + 1) * B], tpk[:], MUL
                    )

                sn_ps = psT.tile([1, HL * B], F32, tag="sn")
                nc.tensor.matmul(sn_ps[:], ones[:], prod[:], start=True, stop=True)
                e_new = wpool.tile([1, HL * B], F32)
                nc.scalar.activation(e_new[:], sn_ps[:], EXP_FN)

            dtot = wpool.tile([1, HL * B], F32)
            rec = wpool.tile([1, HL * B], F32)

            # ---- phase 3: per-request paged attention (slot order) ----
            # Software-pipelined one request ahead: scores(i+1) is issued
            # before attnV(i) so the PE never stalls on the mask->exp->cast
            # round trip through DVE/ACT.
            ph_tiles = {}
            with (
                tc.tile_pool(name="psB", bufs=3, space="PSUM") as psB,
                tc.tile_pool(name="psB2", bufs=2, space="PSUM") as psB2,
            ):
                def do_scores(i):
                    pb = pairs[order[i]]
                    r = rtail[i]          # valid rows in the last pair
                    kvt = kv_tiles[i]
                    # scores^T: [128(s), (h, pair)]
                    scp = psB.tile([128, HL, pb], F32, tag="scp")
                    for h in range(HL):
                        qh = qT_bf[:, h * B + i : h * B + i + 1]
                        for p in range(pb):
                            nc.tensor.matmul(
                                scp[:, h, p : p + 1], kvt[:, p, h, 0, :], qh,
                                start=True, stop=True,
                            )
                    # probs = exp(scores) in bf16 straight off PSUM; rows
                    # >= hist in the last pair are zeroed by an ACT copy with
                    # a per-partition 0/1 scale (same engine, no extra hop)
                    ph = smp.tile([128, HL, pb], BF, tag="ph")
                    nc.scalar.activation(ph[:], scp[:], EXP_FN)
                    if r < 128:
                        nc.scalar.activation(
                            ph[:, :, pb - 1], ph[:, :, pb - 1], COPY_FN,
                            scale=zmask[:, i : i + 1],
                        )
                    ph_tiles[i] = ph

                def do_attnv(i):
                    pb = pairs[order[i]]
                    kvt = kv_tiles.pop(i)
                    ph = ph_tiles.pop(i)
                    # attn^T[d, h] = sum_s p[s] * V[s, d]
                    atp = psB.tile([128, HL], F32, tag="atp")
                    for h in range(HL):
                        for p in range(pb):
                            nc.tensor.matmul(
                                atp[:, h : h + 1], kvt[:, p, h, 1, :],
                                ph[:, h, p : p + 1],
                                start=(p == 0), stop=(p == pb - 1),
                            )
                    nc.scalar.copy(
                        atsb[:].rearrange("d (h b2) -> d h b2", h=HL)[:, :, i], atp[:]
                    )
                    # denominators: column sums of probs
                    dsp = psB2.tile([1, HL * pb], F32, tag="dsp")
                    nc.tensor.matmul(
                        dsp[:], ones[:],
                        ph[:].rearrange("s h p -> s (h p)"),
                        start=True, stop=True,
                    )
                    nc.vector.reduce_sum(
                        dnm[:].rearrange("o (h b2) -> o h b2", h=HL)[:, :, i],
                        dsp[:].rearrange("o (h p) -> o h p", h=HL),
                        axis=mybir.AxisListType.X,
                    )
                    # incremental 1/(denom + e_new) for this slot (tiny, off
                    # the critical tail)
                    dslc = dnm[:].rearrange("o (h b2) -> o h b2", h=HL)[:, :, i]
                    eslc = e_new[:].rearrange("o (h b2) -> o h b2", h=HL)[:, :, i]
                    tslc = dtot[:].rearrange("o (h b2) -> o h b2", h=HL)[:, :, i]
                    rslc = rec[:].rearrange("o (h b2) -> o h b2", h=HL)[:, :, i]
                    nc.vector.tensor_tensor(tslc, dslc, eslc, ADD)
                    nc.vector.reciprocal(rslc, tslc)

                # o_proj weights slot into the KV stream mid-loop; they
                # are only needed at the very end
                wo_sched = {6: 0, 10: 1, 14: 2, 18: 3}
                do_scores(0)
                do_scores(1)
                for i in range(B):
                    if i in wo_sched:
                        issue_wo(wo_sched[i])
                    nxt = i + PRE
                    if nxt < B and nxt not in kv_tiles:
                        load_b(nxt)
                    if i + 2 < B:
                        do_scores(i + 2)
                    do_attnv(i)

            # ---- epilogue: add new token, normalize, project ----
            e_bf = wpool.tile([1, HL * B], BF)
            nc.scalar.copy(e_bf[:], e_new[:])
            r_bf = wpool.tile([1, HL * B], BF)
            nc.scalar.copy(r_bf[:], rec[:])
            att = wpool.tile([128, HL * B], F32)
            with tc.tile_pool(name="psD", bufs=1, space="PSUM") as psD:
                # broadcast rows across partitions via K=1 outer products
                ebp = psD.tile([128, HL * B], F32, tag="ebp")
                nc.tensor.matmul(ebp[:], onesb[:], e_bf[:], start=True, stop=True)
                rbp = psD.tile([128, HL * B], F32, tag="rbp")
                nc.tensor.matmul(rbp[:], onesb[:], r_bf[:], start=True, stop=True)

                nc.vector.tensor_tensor(att[:], vT[:], ebp[:], MUL)
                nc.vector.tensor_tensor(att[:], att[:], atsb[:], ADD)
                nc.vector.tensor_tensor(att[:], att[:], rbp[:], MUL)
            at_bf = wpool.tile([128, HL * B], BF)
            nc.vector.tensor_copy(at_bf[:], att[:])

            with tc.tile_pool(name="psC", bufs=3, space="PSUM") as psC:
                for n in range(8):
                    opsn = psC.tile([B, 512], F32, tag="ops")
                    for h in range(HL):
                        nc.tensor.matmul(
                            opsn[:],
                            at_bf[:, h * B : (h + 1) * B],
                            wo_tiles[h][:, n * 512 : (n + 1) * 512],
                            start=(h == 0),
                            stop=(h == HL - 1),
                        )
                    outc = smp.tile([B, 512], F32, tag="outc")
                    if n % 2:
                        nc.scalar.copy(outc[:], opsn[:])
                    else:
                        nc.vector.tensor_copy(outc[:], opsn[:])
                    nc.sync.dma_start(
                        out=out_part[:, n * 512 : (n + 1) * 512], in_=outc[:]
                    )

    _split_excess_waits(nc)
    return nc


def _host_prep(hidden, W_pack, o_proj_weight, k_cache, v_cache, hist, block_offsets):
    """Build the 8 per-core input maps (numpy only)."""
    hidden = np.asarray(hidden, np.float32)
    W_pack = np.asarray(W_pack, np.float32)
    o_proj_weight = np.asarray(o_proj_weight, np.float32)
    k_cache = np.asarray(k_cache, np.float32)
    v_cache = np.asarray(v_cache, np.float32)
    hist = np.asarray(hist, np.int64)
    block_offsets = np.asarray(block_offsets, np.int64)

    pairs = [int((h + 127) // 128) for h in hist]
    # slot order: descending pairs so the end of the KV stream (which is no
    # longer overlapped with DMA) carries the cheapest compute
    order = sorted(range(B), key=lambda b: (-pairs[b], b))

    # rope tables, scale folded into the q tables
    inv_freq = 1.0 / (ROPE_BASE ** (np.arange(0, D, 2, dtype=np.float32) / D))
    ang = hist.astype(np.float32)[:, None] * inv_freq[None, :]        # [B, 64]
    cos128 = np.concatenate([np.cos(ang), np.cos(ang)], -1)           # [B, 128]
    sin128 = np.concatenate([np.sin(ang), np.sin(ang)], -1)
    sign = np.concatenate([-np.ones(64), np.ones(64)]).astype(np.float32)
    sc = 1.0 / math.sqrt(D)
    tile_h = lambda x: np.tile(x, (1, HL)).astype(np.float32)         # [B, 512]
    cs = np.concatenate(
        [tile_h(cos128 * sc), tile_h(sin128 * sign * sc),
         tile_h(cos128), tile_h(sin128 * sign)], -1,
    )                                                                 # [B, 2048]

    # valid rows in the last loaded pair, per slot (positions < hist)
    rtail = [int(hist[b]) - 128 * (pairs[b] - 1) for b in order]
    zmask = np.zeros((128, B), dtype=np.float32)
    for i, r in enumerate(rtail):
        zmask[:r, i] = 1.0

    hT = np.ascontiguousarray(hidden.T)                               # [4096, 32]
    hT_bf = np.ascontiguousarray(
        hT.astype(BF_NP).reshape(KT, 128, B).transpose(1, 0, 2)
    )                                                                 # [128, KT, B]

    # gather caches via the block table (b-major), slice heads per core
    k_all = k_cache[block_offsets.reshape(-1)]                        # [512,64,32,128]
    v_all = v_cache[block_offsets.reshape(-1)]

    # permutation matrix: column slot i picks original request order[i]
    ident = np.zeros((B, B), dtype=BF_NP)
    ident[np.asarray(order), np.arange(B)] = 1.0

    in_maps = []
    for c in range(NCORES):
        h0 = c * HL
        qcols = np.arange(h0 * D, (h0 + HL) * D)
        wp_c = np.concatenate(
            [W_pack[:, qcols], W_pack[:, HID + qcols], W_pack[:, 2 * HID + qcols]],
            axis=1,
        )                                                             # [4096, 1536]
        wp_bf = wp_c.astype(BF_NP).reshape(KT, 128, 3 * HL * D)

        wo_c = np.ascontiguousarray(o_proj_weight[:, qcols].T)        # [512, 4096]
        wo_bf = wo_c.astype(BF_NP).reshape(HL, 128, HID)

        kc = k_all[:, :, h0 : h0 + HL, :]                             # [512,64,4,128]
        vc = v_all[:, :, h0 : h0 + HL, :]
        # K: [128(d), B, PAIRS, HL, 128(s)]
        kc5 = kc.reshape(B, PAIRS, 2, BS, HL, D)
        kT_c = kc5.transpose(5, 0, 1, 4, 2, 3).reshape(D, B, PAIRS, HL, 128)
        # V: [128(s), B, PAIRS, HL, 128(d)]
        vc5 = vc.reshape(B, PAIRS, 2, BS, HL, D)
        v_c = vc5.transpose(2, 3, 0, 1, 4, 5).reshape(128, B, PAIRS, HL, D)
        # interleave: [...h, 0, :] = K rows, [...h, 1, :] = V rows
        kv_c = np.ascontiguousarray(
            np.stack([kT_c, v_c], axis=4)
        ).astype(BF_NP)                                   # [128,B,PAIRS,HL,2,128]

        in_maps.append({
            "hT": hT_bf, "wp": wp_bf, "wo": wo_bf,
            "kvc": kv_c,
            "cs": cs, "zmask": zmask, "ident": ident,
        })
    return pairs, order, rtail, in_maps


def kernel(hidden_states, W_pack, o_proj_weight, k_cache, v_cache,
           history_lengths, block_offsets):
    global LAST_RESULTS
    pairs, order, rtail, in_maps = _host_prep(
        hidden_states, W_pack, o_proj_weight, k_cache, v_cache,
        history_lengths, block_offsets,
    )
    nc = _build_nc(pairs, order, rtail)
    trace = bool(int(os.environ.get("KERNEL_TRACE", "0")))
    res = run_bass_kernel_spmd(nc, in_maps, list(range(NCORES)), trace=trace)
    LAST_RESULTS = res
    acc = np.zeros((B, HID), np.float32)
    for c in range(NCORES):
        acc += res.results[c]["out_part"]
    out = np.zeros((B, HID), np.float32)
    out[np.asarray(order)] = acc                   # slot rows -> original rows
    return out
